# revision 42
# baseline (speedup 1.0000x reference)
"""Trainium2 Bass kernel for EnhancedMetaWeightNetwork (v2: fp8/bf16).

Full (unsharded) inputs in, full output out. 8 NeuronCores: core c handles
batch b = c // 2 and query-row half c % 2 (SQ=1024 own queries, full S=2048
keys; K/V recomputed per core pair — no collectives).

v2 design (vs fp32r v1):
  - attention path in fp8 e4m3 (QKV/V/AV/dn/out-proj use DoubleRow matmuls,
    256-deep contraction per instruction); scores matmul plain fp8.
  - meta-MLP in bf16 (fp8 there fails the accuracy budget); x stored as 16x
    bf16 with w1's x-half pre-divided by 16 on host.
  - pair-merged exp: scores land in 2-bank [P,2,512] PSUM tiles, one 1024-wide
    exp instruction per pair. exp(score - C_SHIFT) on ACT -> fp8 ex (C_SHIFT
    keeps exp < e4m3 max 240; cancels exactly in softmax). The last
    NKT-KB_NEX_ACT key-tiles per group instead use a Schraudolph exp on DVE
    writing e4m3 bytes directly via uint8 convert (negatives saturate to 0 ==
    zero attention weight), so AV/dn stay DoubleRow everywhere.
  - biases folded on host: V-bias + out_b -> b1_eff (softmax weights sum to
    1); K-bias dropped (adds a per-query constant to scores -> softmax
    cancels it).
  - qb-outer attention groups; once a qb finishes, its out-proj, w1, LN1, h2
    and LN2-final become PE/DVE filler work interleaved into the next qb's
    exp-bound window.
  - PSUM->SBUF stagings split ACT (needs bias: Q, h1p) / DVE (K, V, O);
    softmax normalize + reciprocal_approx_fast on DVE; everything resident in
    SBUF, no DRAM spills.
"""

import os
import numpy as np

H = 1024
NH = 8
HD = 128           # head dim
S = 2048           # keys / full sequence
SQ = 1024          # own query rows per core
MD = 256
MD2 = 128
VOCAB = 32000
MIN_W, MAX_W = 0.1, 5.0
LN_EPS = 1e-5
P = 128
NKT = S // P       # 16 key tiles
NC8 = H // P       # 8 feature chunks
NTT = SQ // P      # 8 own token tiles
NQB = SQ // 512    # 2 query blocks

# fp8 scales
S_X = 16.0
S_W = 1024.0       # fp8 scale for wq/wk/wv/ow (values ~N(0, 0.02^2*1024) -> amax ~0.11)
S_Q = 64.0         # includes 1/sqrt(HD)
S_K = 16.0
S_V = 32.0
S_CTX = 32.0
C_SHIFT = 2.0
SCORE_PS = S_Q * S_K          # score_true * SCORE_PS in PSUM

# Schraudolph bf16 exp constants: bf16 = bitcast(int16(x*128*log2e + (16256-c)))
LOG2E = 1.4426950408889634
SCHR_C = 9.0

_CACHE = {}


def _build(nex_act):
    """nex_act: number of key-tiles per (h,qb) exp'd on ACT (fp8, even);
    the remaining NKT-nex_act use DVE Schraudolph (bf16)."""
    import concourse.bass as bass
    import concourse.mybir as mybir
    import concourse.tile as tile
    from concourse import bacc

    f32 = mybir.dt.float32
    f8 = mybir.dt.float8e4
    bf = mybir.dt.bfloat16
    i16 = mybir.dt.int16
    u8 = mybir.dt.uint8
    i32 = mybir.dt.int32
    OP = mybir.AluOpType
    ACT = mybir.ActivationFunctionType
    DR = mybir.MatmulPerfMode.DoubleRow

    nex_dve = NKT - nex_act
    assert nex_act % 2 == 0 and nex_dve % 2 == 0

    nc = bacc.Bacc("TRN2", target_bir_lowering=False, debug=False,
                   enable_asserts=False, num_devices=8)

    # ---------------- DRAM parameters ----------------
    dp = nc.declare_dram_parameter
    hT = dp("hT", [H, S], bf, isOutput=False)             # hidden[b].T * S_X, own half first
    pT = dp("pT", [H, S], bf, isOutput=False)             # pos.T * S_X
    wq8 = dp("wq8", [P, NC8, H], f8, isOutput=False)      # * s_wq, (c p) n -> p c n
    wk8 = dp("wk8", [P, NC8, H], f8, isOutput=False)
    wv8 = dp("wv8", [P, NC8, H], f8, isOutput=False)
    ow8 = dp("ow8", [P, NC8, H], f8, isOutput=False)
    w1bf = dp("w1bf", [P, 2 * NC8, MD], bf, isOutput=False)
    w2bf = dp("w2bf", [P, MD // P, MD2], bf, isOutput=False)
    bq_c = dp("bq_c", [P, NC8], f32, isOutput=False)      # bq * S_Q/sqrt(HD)
    b1_c = dp("b1_c", [P, MD // P], f32, isOutput=False)  # b1 + w1[:, H:] @ ob_eff
    g1_c = dp("g1_c", [P, MD // P], f32, isOutput=False)
    be1_c = dp("be1_c", [P, MD // P], f32, isOutput=False)
    b2_b = dp("b2_b", [P, MD2], f32, isOutput=False)
    g2_b = dp("g2_b", [P, MD2], f32, isOutput=False)
    be2_b = dp("be2_b", [P, MD2], f32, isOutput=False)
    w3_b = dp("w3_b", [P, MD2], f32, isOutput=False)
    b3_c = dp("b3_c", [P, 1], f32, isOutput=False)
    kb_c = dp("kb_c", [P, NKT], f32, isOutput=False)      # mask bias - C_SHIFT (exp bias)
    kbs_c = dp("kbs_c", [P, NKT], f32, isOutput=False)    # schraudolph additive const
    maskf = dp("maskf", [P, NTT], f32, isOutput=False)
    tok = dp("tok", [SQ, 1], i32, isOutput=False)
    table = dp("table", [VOCAB, 1], f32, isOutput=False)
    out = dp("out", [SQ], f32, isOutput=True)

    with tile.TileContext(nc) as tc:
        with tc.tile_pool(name="const", bufs=1) as cst, \
             tc.tile_pool(name="big", bufs=1) as big:

            # ---------------- constants ----------------
            ones8 = cst.tile([P, 2, P], f8, tag="ones8")
            nc.any.memset(ones8[:], 1.0)
            onesbf = cst.tile([P, P], bf, tag="onesbf")
            nc.any.memset(onesbf[:], 1.0)
            eps_sb = cst.tile([P, 1], f32, tag="eps")
            nc.any.memset(eps_sb[:], LN_EPS)

            def cload(shape, tag, src, dt=f32):
                t = cst.tile(shape, dt, tag=tag)
                nc.sync.dma_start(t[:], src[:])
                return t

            kb_sb = cload([P, NKT], "kb", kb_c)
            kbs_sb = cload([P, NKT], "kbs", kbs_c)
            maskf_sb = cload([P, NTT], "maskf", maskf)
            b3_sb = cload([P, 1], "b3", b3_c)
            w3_sb = cload([P, MD2], "w3", w3_b)
            bq_sb = cload([P, NC8], "bq", bq_c)
            b1_sb = cload([P, MD // P], "b1c", b1_c)
            g1_sb = cload([P, MD // P], "g1c", g1_c)
            be1_sb = cload([P, MD // P], "be1c", be1_c)
            b2_sb = cload([P, MD2], "b2", b2_b)
            g2_sb = cload([P, MD2], "g2", g2_b)
            be2_sb = cload([P, MD2], "be2", be2_b)

            imp_all = cst.tile([P, NTT], f32, tag="imp_all")
            itt_all = cst.tile([P, NTT], i32, tag="itt_all")
            nc.sync.dma_start(itt_all[:],
                              tok[:, 0].rearrange("(t p) -> p t", p=P))
            for tt in range(NTT):
                nc.gpsimd.indirect_dma_start(
                    out=imp_all[:, tt:tt + 1], out_offset=None, in_=table[:],
                    in_offset=bass.IndirectOffsetOnAxis(ap=itt_all[:, tt:tt + 1],
                                                        axis=0))

            # persistent activations
            x8 = big.tile([P, NC8, S], f8, tag="x8")          # x * S_X
            xbf = big.tile([P, NC8, SQ], bf, tag="xbf")       # x own half, scale 1
            q8 = big.tile([P, NC8, SQ], f8, tag="q8")         # q * S_Q/sqrt(HD), feat-major
            k8 = big.tile([P, NC8, S], f8, tag="k8")          # k * S_K
            v8 = big.tile([P, NKT, H], f8, tag="v8")          # v * S_V, token-major
            ctx8 = big.tile([P, NC8, SQ], f8, tag="ctx8")     # ctx * S_CTX, feat-major
            attbf = big.tile([P, NC8, SQ], bf, tag="attbf")   # attended, scale 1

            # ---------- phase X ----------
            # host sends h,p pre-scaled by S_X; xbf holds 16x (w1's x-half is
            # divided by 16 on host to compensate). Other half: add -> fp8
            # directly; own half: add -> bf16, then ACT copy -> fp8.
            with tc.tile_pool(name="tmpx", bufs=2) as tmp:
                for c4 in range(NC8 // 2):
                    ht4 = tmp.tile([P, 2, S], bf, tag="ht4")
                    pt4 = tmp.tile([P, 2, S], bf, tag="pt4")
                    nc.sync.dma_start(
                        ht4[:], hT[c4 * 256:(c4 + 1) * 256, :]
                        .rearrange("(c p) s -> p c s", p=P))
                    nc.sync.dma_start(
                        pt4[:], pT[c4 * 256:(c4 + 1) * 256, :]
                        .rearrange("(c p) s -> p c s", p=P))
                    for i in range(2):
                        c8 = 2 * c4 + i
                        nc.vector.tensor_tensor(out=xbf[:, c8, :],
                                                in0=ht4[:, i, 0:SQ],
                                                in1=pt4[:, i, 0:SQ], op=OP.add)
                        nc.scalar.activation(x8[:, c8, 0:SQ], xbf[:, c8, :],
                                             ACT.Identity, bias=0.0, scale=1.0)
                        nc.vector.tensor_tensor(out=x8[:, c8, SQ:S],
                                                in0=ht4[:, i, SQ:S],
                                                in1=pt4[:, i, SQ:S], op=OP.add)

            # weights
            wq_sb = cload([P, NC8, H], "wq", wq8, f8)
            wk_sb = cload([P, NC8, H], "wk", wk8, f8)
            wv_sb = cload([P, NC8, H], "wv", wv8, f8)
            ow_sb = cload([P, NC8, H], "ow", ow8, f8)
            w1_sb = cload([P, 2 * NC8, MD], "w1", w1bf, bf)
            w2_sb = cload([P, MD // P, MD2], "w2", w2bf, bf)

            # ---------- phase QKV (pair-merged PSUM tiles, 1024-wide stagings) ----------
            with tc.tile_pool(name="ps_mm1", bufs=3, space="PSUM") as ps1:
                # Q/K only for head 0 here; heads 1..7 are emitted inside
                # the attention loop (emit_qk) so exp can start ~40us earlier.
                def emit_qk_with(pool, tag, h):
                    psq = pool.tile([P, 2, 512], f32, tag=tag, name="psq")
                    for j in range(NC8 // 2):
                        for qb in range(NQB):
                            nc.tensor.matmul(
                                psq[:, qb, :],
                                lhsT=wq_sb[:, 2 * j:2 * j + 2, h * P:(h + 1) * P],
                                rhs=x8[:, 2 * j:2 * j + 2, qb * 512:(qb + 1) * 512],
                                start=(j == 0), stop=(j == NC8 // 2 - 1),
                                perf_mode=DR)
                    nc.scalar.activation(
                        q8[:, h, :], psq[:], ACT.Identity,
                        bias=bq_sb[:, h:h + 1],
                        scale=S_Q / (np.sqrt(HD) * S_X * S_W))
                    for sp in range(2):
                        psk = pool.tile([P, 2, 512], f32, tag=tag, name="psk")
                        for j in range(NC8 // 2):
                            for i in range(2):
                                sb = 2 * sp + i
                                nc.tensor.matmul(
                                    psk[:, i, :],
                                    lhsT=wk_sb[:, 2 * j:2 * j + 2, h * P:(h + 1) * P],
                                    rhs=x8[:, 2 * j:2 * j + 2, sb * 512:(sb + 1) * 512],
                                    start=(j == 0), stop=(j == NC8 // 2 - 1),
                                    perf_mode=DR)
                        nc.vector.tensor_scalar(
                            k8[:, h, sp * 1024:(sp + 1) * 1024], psk[:],
                            S_K / (S_X * S_W), None, op0=OP.mult)

                emit_qk_with(ps1, "mmp", 0)
                # V token-major [tok, feat], no bias (folded into b1_eff)
                for tt in range(NKT):
                    psv = ps1.tile([P, 2, 512], f32, tag="mmp", name="psv")
                    for j in range(NC8 // 2):
                        for db in range(2):
                            nc.tensor.matmul(
                                psv[:, db, :],
                                lhsT=x8[:, 2 * j:2 * j + 2, tt * P:(tt + 1) * P],
                                rhs=wv_sb[:, 2 * j:2 * j + 2, db * 512:(db + 1) * 512],
                                start=(j == 0), stop=(j == NC8 // 2 - 1),
                                perf_mode=DR)
                    nc.vector.tensor_scalar(
                        v8[:, tt, :], psv[:], S_V / (S_X * S_W), None, op0=OP.mult)

            # ---------- attention + out-proj + w1, interleaved ----------
            # qb-outer group order; once a qb's 8 head-groups are done, its
            # out-proj and w1 matmuls become PE filler work emitted between
            # the next groups (the ACT engine is the bottleneck during exp).
            NFT = MD // P      # 2 feature tiles of h1
            with tc.tile_pool(name="exs", bufs=2) as exs, \
                 tc.tile_pool(name="asml", bufs=2) as asml, \
                 tc.tile_pool(name="mw", bufs=1) as mw, \
                 tc.tile_pool(name="msml", bufs=1) as sml, \
                 tc.tile_pool(name="ps_sc", bufs=2, space="PSUM") as ps_sc, \
                 tc.tile_pool(name="ps_ctx", bufs=1, space="PSUM") as ps_ctx, \
                 tc.tile_pool(name="ps_dn", bufs=1, space="PSUM") as ps_dn, \
                 tc.tile_pool(name="ps_io", bufs=2, space="PSUM") as ps_io:
                h1p = mw.tile([P, NFT, SQ], bf, tag="h1p")
                h1sq = mw.tile([P, NFT, SQ], bf, tag="h1sq")
                h1n = mw.tile([P, NFT, SQ], bf, tag="h1n")
                stat = mw.tile([P, 3, SQ], f32, tag="stat")
                res_sb = mw.tile([P, NTT], f32, tag="res")
                nmean, work, m2r = stat[:, 0, :], stat[:, 1, :], stat[:, 2, :]
                ex2m = varm = rstd = work

                def emit_oproj(qb, dt):
                    qsl = slice(qb * 512, (qb + 1) * 512)
                    pso = ps_io.tile([P, 512], f32, tag="psio", name="pso")
                    for j in range(NC8 // 2):
                        nc.tensor.matmul(
                            pso[:],
                            lhsT=ow_sb[:, 2 * j:2 * j + 2, dt * P:(dt + 1) * P],
                            rhs=ctx8[:, 2 * j:2 * j + 2, qsl],
                            start=(j == 0), stop=(j == NC8 // 2 - 1),
                            perf_mode=DR)
                    # bias (incl. folded V bias) lives in b1_eff on host
                    nc.vector.tensor_scalar(
                        attbf[:, dt, qsl], pso[:],
                        1.0 / (S_CTX * S_W), None, op0=OP.mult)

                def emit_w1(qb, ft):
                    qsl = slice(qb * 512, (qb + 1) * 512)
                    psf = ps_io.tile([P, 512], f32, tag="psio", name="psf")
                    for c16 in range(2 * NC8):
                        if c16 < NC8:
                            rhs = xbf[:, c16, qsl]
                        else:
                            rhs = attbf[:, c16 - NC8, qsl]
                        nc.tensor.matmul(
                            psf[:],
                            lhsT=w1_sb[:, c16, ft * P:(ft + 1) * P],
                            rhs=rhs,
                            start=(c16 == 0), stop=(c16 == 2 * NC8 - 1))
                    nc.scalar.activation(
                        h1p[:, ft, qsl], psf[:],
                        ACT.Identity, bias=b1_sb[:, ft:ft + 1], scale=1.0)

                hb2_all = mw.tile([P, NTT, MD2], f32, tag="hb2_all")

                def emit_ln1(qb):
                    qsl = slice(qb * 512, (qb + 1) * 512)
                    for ft in range(NFT):
                        nc.vector.tensor_tensor(out=h1sq[:, ft, qsl],
                                                in0=h1p[:, ft, qsl],
                                                in1=h1p[:, ft, qsl], op=OP.mult)
                    psA = ps_io.tile([P, 512], f32, tag="psio", name="psA")
                    psB = ps_io.tile([P, 512], f32, tag="psio", name="psB")
                    for ft in range(NFT):
                        nc.tensor.matmul(psA[:], lhsT=onesbf[:],
                                         rhs=h1p[:, ft, qsl],
                                         start=(ft == 0), stop=(ft == NFT - 1))
                    for ft in range(NFT):
                        nc.tensor.matmul(psB[:], lhsT=onesbf[:],
                                         rhs=h1sq[:, ft, qsl],
                                         start=(ft == 0), stop=(ft == NFT - 1))
                    nc.vector.tensor_scalar_mul(nmean[:, qsl], psA[:], -1.0 / MD)
                    nc.vector.tensor_scalar_mul(ex2m[:, qsl], psB[:], 1.0 / MD)
                    nc.vector.tensor_tensor(out=m2r[:, qsl], in0=nmean[:, qsl],
                                            in1=nmean[:, qsl], op=OP.mult)
                    nc.vector.tensor_tensor(out=work[:, qsl], in0=work[:, qsl],
                                            in1=m2r[:, qsl], op=OP.subtract)
                    # rstd = exp(-0.5 * ln(var + eps))
                    nc.scalar.activation(varm[:, qsl], varm[:, qsl], ACT.Ln,
                                         bias=eps_sb[:, 0:1], scale=1.0)
                    nc.scalar.activation(rstd[:, qsl], varm[:, qsl], ACT.Exp,
                                         bias=0.0, scale=-0.5)
                    for ft in range(NFT):
                        nc.vector.tensor_tensor(out=h1n[:, ft, qsl],
                                                in0=h1p[:, ft, qsl],
                                                in1=nmean[:, qsl], op=OP.add)
                        nc.vector.tensor_tensor(out=h1n[:, ft, qsl],
                                                in0=h1n[:, ft, qsl],
                                                in1=rstd[:, qsl], op=OP.mult)
                        nc.scalar.activation(h1n[:, ft, qsl], h1n[:, ft, qsl],
                                             ACT.Relu, bias=be1_sb[:, ft:ft + 1],
                                             scale=g1_sb[:, ft:ft + 1])

                def emit_h2(tt, _unused=None):
                    ph2_t = ps_io.tile([P, 512], f32, tag="psio", name="ph2")
                    ph2 = ph2_t[:, :MD2]
                    for ft in range(MD // P):
                        nc.tensor.matmul(ph2, lhsT=h1n[:, ft, tt * P:(tt + 1) * P],
                                         rhs=w2_sb[:, ft, :],
                                         start=(ft == 0), stop=(ft == MD // P - 1))
                    nc.vector.scalar_tensor_tensor(out=hb2_all[:, tt, :], in0=ph2,
                                                   scalar=1.0, in1=b2_sb[:],
                                                   op0=OP.mult, op1=OP.add)

                F2 = float(MD2)

                def emit_tail(qb, _unused=None):
                    r = slice(qb * NTT // NQB, (qb + 1) * NTT // NQB)
                    nt = NTT // NQB
                    hb2 = hb2_all[:, r, :]
                    sums2 = sml.tile([P, NTT], f32, tag="sums2")
                    nc.vector.reduce_sum(sums2[:, r], hb2,
                                         axis=mybir.AxisListType.X)
                    msq = sml.tile([P, NTT, MD2], f32, tag="msq")
                    ssq2 = sml.tile([P, NTT], f32, tag="ssq2")
                    nc.vector.tensor_tensor(out=msq[:, r, :], in0=hb2,
                                            in1=hb2, op=OP.mult)
                    nc.vector.reduce_sum(ssq2[:, r], msq[:, r, :],
                                         axis=mybir.AxisListType.X)
                    nm2 = sml.tile([P, NTT], f32, tag="nm2")
                    nc.vector.tensor_scalar_mul(nm2[:, r], sums2[:, r], -1.0 / F2)
                    ex22 = sml.tile([P, NTT], f32, tag="ex22")
                    nc.vector.tensor_scalar_mul(ex22[:, r], ssq2[:, r], 1.0 / F2)
                    mm2 = sml.tile([P, NTT], f32, tag="mm2")
                    nc.vector.tensor_tensor(out=mm2[:, r], in0=nm2[:, r],
                                            in1=nm2[:, r], op=OP.mult)
                    var2 = sml.tile([P, NTT], f32, tag="var2")
                    nc.vector.tensor_tensor(out=var2[:, r], in0=ex22[:, r],
                                            in1=mm2[:, r], op=OP.subtract)
                    std2 = sml.tile([P, NTT], f32, tag="std2")
                    nc.scalar.activation(std2[:, r], var2[:, r], ACT.Sqrt,
                                         bias=eps_sb[:, 0:1], scale=1.0)
                    rstd2 = sml.tile([P, NTT], f32, tag="rstd2")
                    nc.vector.reciprocal_approx_fast(rstd2[:, r], std2[:, r])
                    t1a = sml.tile([P, NTT, MD2], f32, tag="t1a")
                    nc.vector.tensor_tensor(
                        out=t1a[:, r, :], in0=hb2,
                        in1=nm2[:, r, None].to_broadcast([P, nt, MD2]), op=OP.add)
                    nc.vector.tensor_tensor(
                        out=t1a[:, r, :], in0=t1a[:, r, :],
                        in1=rstd2[:, r, None].to_broadcast([P, nt, MD2]),
                        op=OP.mult)
                    nc.vector.tensor_tensor(
                        out=t1a[:, r, :], in0=t1a[:, r, :],
                        in1=g2_sb[:, None, :].to_broadcast([P, nt, MD2]),
                        op=OP.mult)
                    nc.vector.tensor_tensor(
                        out=t1a[:, r, :], in0=t1a[:, r, :],
                        in1=be2_sb[:, None, :].to_broadcast([P, nt, MD2]),
                        op=OP.add)
                    nc.vector.tensor_scalar_max(t1a[:, r, :], t1a[:, r, :], 0.0)
                    nc.vector.tensor_tensor(
                        out=t1a[:, r, :], in0=t1a[:, r, :],
                        in1=w3_sb[:, None, :].to_broadcast([P, nt, MD2]),
                        op=OP.mult)
                    base8 = sml.tile([P, NTT], f32, tag="base8")
                    nc.vector.reduce_sum(base8[:, r], t1a[:, r, :],
                                         axis=mybir.AxisListType.X)
                    nc.vector.tensor_tensor(
                        out=base8[:, r], in0=base8[:, r],
                        in1=b3_sb[:, 0:1].to_broadcast([P, nt]), op=OP.add)
                    imp1a = sml.tile([P, NTT], f32, tag="imp1a")
                    nc.vector.tensor_scalar_add(imp1a[:, r], imp_all[:, r], 1.0)
                    nc.vector.tensor_tensor(out=base8[:, r], in0=base8[:, r],
                                            in1=imp1a[:, r], op=OP.mult)
                    nc.vector.tensor_scalar(base8[:, r], base8[:, r], MAX_W, MIN_W,
                                            op0=OP.min, op1=OP.max)
                    nc.vector.tensor_tensor(out=res_sb[:, r], in0=base8[:, r],
                                            in1=maskf_sb[:, r], op=OP.mult)
                    nc.sync.dma_start(
                        out[:].rearrange("(t p) -> p t", p=P)[:, r], res_sb[:, r])

                groups = [(h, qb) for qb in range(NQB) for h in range(NH)]
                NPAIR = NKT // 2
                fillers = []

                def emit_scexp(g, ex8):
                    h, qb = groups[g]
                    qsl = slice(qb * 512, (qb + 1) * 512)
                    for t in range(NPAIR):
                        scp = ps_sc.tile([P, 2, 512], f32, tag="scp")
                        for i in range(2):
                            nc.tensor.matmul(scp[:, i, :],
                                             lhsT=k8[:, h, (2 * t + i) * P:
                                                  (2 * t + i + 1) * P],
                                             rhs=q8[:, h, qsl],
                                             start=True, stop=True)
                        if 2 * t < nex_act:
                            nc.scalar.activation(ex8[:, 2 * t:2 * t + 2, :], scp[:],
                                                 ACT.Exp,
                                                 bias=kb_sb[:, 2 * t:2 * t + 1],
                                                 scale=1.0 / SCORE_PS)
                        else:
                            # Schraudolph exp straight to e4m3: uint8 convert
                            # saturates negatives to 0 (== zero attn weight)
                            nc.vector.tensor_scalar(
                                ex8[:, 2 * t:2 * t + 2, :].bitcast(u8),
                                scp[:], 8.0 * LOG2E / SCORE_PS,
                                kbs_sb[:, 2 * t:2 * t + 1],
                                op0=OP.mult, op1=OP.add)

                ex_tiles = {}

                def alloc_ex():
                    ex8 = exs.tile([P, NKT, 512], f8, tag="ex8", name="ex8")
                    return (ex8,)

                ex_tiles[0] = alloc_ex()
                emit_scexp(0, *ex_tiles[0])
                for g, (h, qb) in enumerate(groups):
                    qsl = slice(qb * 512, (qb + 1) * 512)
                    ex8, = ex_tiles.pop(g)
                    if g + 1 < len(groups):
                        nh_, nqb_ = groups[g + 1]
                        if nqb_ == 0 and nh_ > 0:
                            emit_qk_with(ps_sc, "scp", nh_)
                        ex_tiles[g + 1] = alloc_ex()
                        emit_scexp(g + 1, *ex_tiles[g + 1])
                    cps = ps_ctx.tile([P, 512], f32, tag="cps")
                    dn = ps_dn.tile([P, 512], f32, tag="dn")
                    n_mm = NKT // 2
                    for t in range(n_mm):
                        nc.tensor.matmul(cps[:],
                                         lhsT=v8[:, 2 * t:2 * t + 2, h * P:(h + 1) * P],
                                         rhs=ex8[:, 2 * t:2 * t + 2, :],
                                         start=(t == 0), stop=(t == n_mm - 1),
                                         perf_mode=DR)
                        nc.tensor.matmul(dn[:], lhsT=ones8[:],
                                         rhs=ex8[:, 2 * t:2 * t + 2, :],
                                         start=(t == 0), stop=(t == n_mm - 1),
                                         perf_mode=DR)
                    rcb = asml.tile([P, 512], f32, tag="rcb")
                    nc.vector.reciprocal_approx_fast(rcb[:], dn[:])
                    nc.vector.scalar_tensor_tensor(
                        out=ctx8[:, h, qsl], in0=cps[:], scalar=S_CTX / S_V,
                        in1=rcb[:], op0=OP.mult, op1=OP.mult)
                    if h == NH - 1:
                        fillers += [(emit_oproj, qb, dt) for dt in range(NC8)]
                        fillers += [(emit_w1, qb, ft) for ft in range(NFT)]
                        fillers += [(lambda q, _u: emit_ln1(q), qb, None)]
                        fillers += [(emit_h2, tt, None)
                                    for tt in range(qb * NTT // NQB,
                                                    (qb + 1) * NTT // NQB)]
                        fillers += [(emit_tail, qb, None)]
                    for _ in range(2):
                        if fillers:
                            fn, a, b2_ = fillers.pop(0)
                            fn(a, b2_)
                while fillers:
                    fn, a, b2_ = fillers.pop(0)
                    fn(a, b2_)

                # LN2/final emitted per-qb as fillers (emit_tail)

    nc.compile()
    return nc


def _get_program():
    nex_act = int(os.environ.get("KB_NEX_ACT", "14"))
    key = ("nc", nex_act)
    if key not in _CACHE:
        _CACHE[key] = _build(nex_act)
    return _CACHE[key]


def _prep_in_maps(inputs):
    import ml_dtypes
    bf16 = ml_dtypes.bfloat16
    f8 = ml_dtypes.float8_e4m3

    hidden = np.asarray(inputs["hidden_states"], dtype=np.float32)
    token_ids = np.asarray(inputs["token_ids"], dtype=np.int32)
    mask = np.asarray(inputs["attention_mask"]).astype(bool)
    pos = np.asarray(inputs["pos_embed"], dtype=np.float32)
    in_proj_w = np.asarray(inputs["in_proj_w"], dtype=np.float32)
    in_proj_b = np.asarray(inputs["in_proj_b"], dtype=np.float32)
    out_w = np.asarray(inputs["out_w"], dtype=np.float32)
    out_b = np.asarray(inputs["out_b"], dtype=np.float32)
    w1 = np.asarray(inputs["w1"], dtype=np.float32)
    b1 = np.asarray(inputs["b1"], dtype=np.float32)
    g1 = np.asarray(inputs["g1"], dtype=np.float32)
    beta1 = np.asarray(inputs["beta1"], dtype=np.float32)
    w2 = np.asarray(inputs["w2"], dtype=np.float32)
    b2 = np.asarray(inputs["b2"], dtype=np.float32)
    g2 = np.asarray(inputs["g2"], dtype=np.float32)
    beta2 = np.asarray(inputs["beta2"], dtype=np.float32)
    w3 = np.asarray(inputs["w3"], dtype=np.float32)
    b3 = np.asarray(inputs["b3"], dtype=np.float32)
    table = np.asarray(inputs["importance_table"], dtype=np.float32)

    B, S_, H_ = hidden.shape
    assert (B, S_, H_) == (4, S, H), (B, S_, H_)

    posT = np.ascontiguousarray(pos[0].T)                      # [H, S]
    wqT = in_proj_w[0:H].T
    wkT = in_proj_w[H:2 * H].T
    wvT = in_proj_w[2 * H:3 * H].T
    bq = in_proj_b[0:H]
    bk = in_proj_b[H:2 * H]
    bv = in_proj_b[2 * H:3 * H]
    owT = out_w.T
    # softmax weights sum to 1, so the V bias passes through attention as a
    # constant: fold it (and out_b) into the out-proj bias, then fold that
    # into the w1 bias (b1_eff), since attended only feeds w1.
    ob_eff = bv @ out_w.T + out_b
    b1_eff = b1 + w1[:, H:2 * H] @ ob_eff

    def warr(wT, s=S_W):   # [H, N] -> [128, 8, N] fp8 scaled
        return np.ascontiguousarray(
            np.clip(wT * s, -224.0, 224.0)
            .reshape(NC8, P, -1).transpose(1, 0, 2)).astype(f8)

    def cmaj(v, s=1.0):   # [F] -> [128, F/128]
        return np.ascontiguousarray((v * s).reshape(-1, P).T.astype(np.float32))

    def bcast(v):  # [F] -> [128, F]
        return np.ascontiguousarray(
            np.broadcast_to(v[None, :], (P, v.shape[0])).astype(np.float32))

    shared = {
        "wq8": warr(wqT), "wk8": warr(wkT),
        "wv8": warr(wvT), "ow8": warr(owT),
        # x is stored on-chip as 16x (bf16); compensate in w1's x-half
        "w1bf": np.ascontiguousarray(
            np.concatenate([w1.T[:H] / S_X, w1.T[H:]], axis=0)
            .reshape(2 * NC8, P, MD).transpose(1, 0, 2)).astype(bf16),
        "w2bf": np.ascontiguousarray(
            w2.T.reshape(MD // P, P, MD2).transpose(1, 0, 2)).astype(bf16),
        "bq_c": cmaj(bq, S_Q / np.sqrt(HD)),
        "b1_c": cmaj(b1_eff), "g1_c": cmaj(g1), "be1_c": cmaj(beta1),
        "b2_b": bcast(b2), "g2_b": bcast(g2), "be2_b": bcast(beta2),
        "w3_b": bcast(w3[0]), "b3_c": np.full((P, 1), b3[0], dtype=np.float32),
        "table": np.ascontiguousarray(table[:, None]),
    }
    in_maps = []
    for c in range(8):
        b = c // 2
        half = c % 2
        own = slice(half * SQ, (half + 1) * SQ)
        oth = slice((1 - half) * SQ, (2 - half) * SQ)
        hT_b = hidden[b].T * S_X
        posT_s = posT * S_X
        hT_arr = np.ascontiguousarray(
            np.concatenate([hT_b[:, own], hT_b[:, oth]], axis=1)).astype(bf16)
        pT_arr = np.ascontiguousarray(
            np.concatenate([posT_s[:, own], posT_s[:, oth]], axis=1)).astype(bf16)
        kb = np.where(mask[b], 0.0, -1e9).astype(np.float32)
        kb_arr = np.concatenate([kb[own], kb[oth]]) - C_SHIFT
        kbs_arr = (56.0 - 0.5) + 8.0 * LOG2E * (
            np.concatenate([kb[own], kb[oth]]) - C_SHIFT)
        m = {
            "hT": hT_arr, "pT": pT_arr,
            "kb_c": np.ascontiguousarray(kb_arr.reshape(-1, P).T),
            "kbs_c": np.ascontiguousarray(
                kbs_arr.reshape(-1, P).T.astype(np.float32)),
            "maskf": np.ascontiguousarray(
                mask[b, own].astype(np.float32).reshape(-1, P).T),
            "tok": np.ascontiguousarray(token_ids[b, own][:, None]),
        }
        m.update(shared)
        in_maps.append(m)
    return in_maps


def _assemble(res):
    full = np.zeros((4, S), dtype=np.float32)
    for c in range(8):
        b = c // 2
        half = c % 2
        full[b, half * SQ:(half + 1) * SQ] = res.results[c]["out"]
    return full


def kernel(**inputs) -> np.ndarray:
    from concourse.bass_utils import run_bass_kernel_spmd
    in_maps = _prep_in_maps(inputs)
    nc = _get_program()
    res = run_bass_kernel_spmd(nc, in_maps, list(range(8)))
    return _assemble(res)


def run_traced(inputs, **kwargs):
    from concourse.bass_utils import run_bass_kernel_spmd
    in_maps = _prep_in_maps(inputs)
    nc = _get_program()
    return run_bass_kernel_spmd(nc, in_maps, list(range(8)), trace=True, **kwargs)


# revision 43
# speedup vs baseline: 1.0286x; 1.0286x over previous
"""Trainium2 Bass kernel for EnhancedMetaWeightNetwork (v2: fp8/bf16).

Full (unsharded) inputs in, full output out. 8 NeuronCores: core c handles
batch b = c // 2 and query-row half c % 2 (SQ=1024 own queries, full S=2048
keys; K/V recomputed per core pair — no collectives).

v2 design (vs fp32r v1):
  - attention path in fp8 e4m3 (QKV/V/AV/dn/out-proj use DoubleRow matmuls,
    256-deep contraction per instruction); scores matmul plain fp8.
  - meta-MLP in bf16 (fp8 there fails the accuracy budget); x stored as 16x
    bf16 with w1's x-half pre-divided by 16 on host.
  - pair-merged exp: scores land in 2-bank [P,2,512] PSUM tiles, one 1024-wide
    exp instruction per pair. exp(score - C_SHIFT) on ACT -> fp8 ex (C_SHIFT
    keeps exp < e4m3 max 240; cancels exactly in softmax). The last
    NKT-KB_NEX_ACT key-tiles per group instead use a Schraudolph exp on DVE
    writing e4m3 bytes directly via uint8 convert (negatives saturate to 0 ==
    zero attention weight), so AV/dn stay DoubleRow everywhere.
  - biases folded on host: V-bias + out_b -> b1_eff (softmax weights sum to
    1); K-bias dropped (adds a per-query constant to scores -> softmax
    cancels it).
  - qb-outer attention groups; once a qb finishes, its out-proj, w1, LN1, h2
    and LN2-final become PE/DVE filler work interleaved into the next qb's
    exp-bound window.
  - PSUM->SBUF stagings split ACT (needs bias: Q, h1p) / DVE (K, V, O);
    softmax normalize + reciprocal_approx_fast on DVE; everything resident in
    SBUF, no DRAM spills.
"""

import os
import numpy as np

H = 1024
NH = 8
HD = 128           # head dim
S = 2048           # keys / full sequence
SQ = 1024          # own query rows per core
MD = 256
MD2 = 128
VOCAB = 32000
MIN_W, MAX_W = 0.1, 5.0
LN_EPS = 1e-5
P = 128
NKT = S // P       # 16 key tiles
NC8 = H // P       # 8 feature chunks
NTT = SQ // P      # 8 own token tiles
NQB = SQ // 512    # 2 query blocks

# fp8 scales
S_X = 16.0
S_W = 1024.0       # fp8 scale for wq/wk/wv/ow (values ~N(0, 0.02^2*1024) -> amax ~0.11)
S_Q = 64.0         # includes 1/sqrt(HD)
S_K = 16.0
S_V = 32.0
S_CTX = 32.0
C_SHIFT = 2.0
SCORE_PS = S_Q * S_K          # score_true * SCORE_PS in PSUM

# Schraudolph bf16 exp constants: bf16 = bitcast(int16(x*128*log2e + (16256-c)))
LOG2E = 1.4426950408889634
SCHR_C = 9.0

_CACHE = {}


def _build(nex_act):
    """nex_act: number of key-tiles per (h,qb) exp'd on ACT (fp8, even);
    the remaining NKT-nex_act use DVE Schraudolph (bf16)."""
    import concourse.bass as bass
    import concourse.mybir as mybir
    import concourse.tile as tile
    from concourse import bacc

    f32 = mybir.dt.float32
    f8 = mybir.dt.float8e4
    bf = mybir.dt.bfloat16
    i16 = mybir.dt.int16
    u8 = mybir.dt.uint8
    i32 = mybir.dt.int32
    OP = mybir.AluOpType
    ACT = mybir.ActivationFunctionType
    DR = mybir.MatmulPerfMode.DoubleRow

    nex_dve = NKT - nex_act
    assert nex_act % 2 == 0 and nex_dve % 2 == 0

    nc = bacc.Bacc("TRN2", target_bir_lowering=False, debug=False,
                   enable_asserts=False, num_devices=8)

    # ---------------- DRAM parameters ----------------
    dp = nc.declare_dram_parameter
    hT = dp("hT", [H, S], bf, isOutput=False)             # hidden[b].T * S_X, own half first
    pT = dp("pT", [H, S], bf, isOutput=False)             # pos.T * S_X
    wq8 = dp("wq8", [P, NC8, H], f8, isOutput=False)      # * s_wq, (c p) n -> p c n
    wk8 = dp("wk8", [P, NC8, H], f8, isOutput=False)
    wv8 = dp("wv8", [P, NC8, H], f8, isOutput=False)
    ow8 = dp("ow8", [P, NC8, H], f8, isOutput=False)
    w1bf = dp("w1bf", [P, 2 * NC8, MD], bf, isOutput=False)
    w2bf = dp("w2bf", [P, MD // P, MD2], bf, isOutput=False)
    bq_c = dp("bq_c", [P, NC8], f32, isOutput=False)      # bq * S_Q/sqrt(HD)
    b1_c = dp("b1_c", [P, MD // P], f32, isOutput=False)  # b1 + w1[:, H:] @ ob_eff
    g1_c = dp("g1_c", [P, MD // P], f32, isOutput=False)
    be1_c = dp("be1_c", [P, MD // P], f32, isOutput=False)
    b2_b = dp("b2_b", [P, MD2], f32, isOutput=False)
    g2_b = dp("g2_b", [P, MD2], f32, isOutput=False)
    be2_b = dp("be2_b", [P, MD2], f32, isOutput=False)
    w3_b = dp("w3_b", [P, MD2], f32, isOutput=False)
    b3_c = dp("b3_c", [P, 1], f32, isOutput=False)
    kb_c = dp("kb_c", [P, NKT], f32, isOutput=False)      # mask bias - C_SHIFT (exp bias)
    kbs_c = dp("kbs_c", [P, NKT], f32, isOutput=False)    # schraudolph additive const
    maskf = dp("maskf", [P, NTT], f32, isOutput=False)
    tok = dp("tok", [SQ, 1], i32, isOutput=False)
    table = dp("table", [VOCAB, 1], f32, isOutput=False)
    out = dp("out", [SQ], f32, isOutput=True)

    with tile.TileContext(nc) as tc:
        with tc.tile_pool(name="const", bufs=1) as cst, \
             tc.tile_pool(name="big", bufs=1) as big:

            # ---------------- constants ----------------
            ones8 = cst.tile([P, 2, P], f8, tag="ones8")
            nc.any.memset(ones8[:], 1.0)
            onesbf = cst.tile([P, P], bf, tag="onesbf")
            nc.any.memset(onesbf[:], 1.0)
            eps_sb = cst.tile([P, 1], f32, tag="eps")
            nc.any.memset(eps_sb[:], LN_EPS)

            def cload(shape, tag, src, dt=f32):
                t = cst.tile(shape, dt, tag=tag)
                nc.sync.dma_start(t[:], src[:])
                return t

            kb_sb = cload([P, NKT], "kb", kb_c)
            kbs_sb = cload([P, NKT], "kbs", kbs_c)
            maskf_sb = cload([P, NTT], "maskf", maskf)
            b3_sb = cload([P, 1], "b3", b3_c)
            w3_sb = cload([P, MD2], "w3", w3_b)
            bq_sb = cload([P, NC8], "bq", bq_c)
            b1_sb = cload([P, MD // P], "b1c", b1_c)
            g1_sb = cload([P, MD // P], "g1c", g1_c)
            be1_sb = cload([P, MD // P], "be1c", be1_c)
            b2_sb = cload([P, MD2], "b2", b2_b)
            g2_sb = cload([P, MD2], "g2", g2_b)
            be2_sb = cload([P, MD2], "be2", be2_b)

            imp_all = cst.tile([P, NTT], f32, tag="imp_all")
            itt_all = cst.tile([P, NTT], i32, tag="itt_all")
            nc.sync.dma_start(itt_all[:],
                              tok[:, 0].rearrange("(t p) -> p t", p=P))
            for tt in range(NTT):
                nc.gpsimd.indirect_dma_start(
                    out=imp_all[:, tt:tt + 1], out_offset=None, in_=table[:],
                    in_offset=bass.IndirectOffsetOnAxis(ap=itt_all[:, tt:tt + 1],
                                                        axis=0))

            # persistent activations
            x8 = big.tile([P, NC8, S], f8, tag="x8")          # x * S_X
            xbf = big.tile([P, NC8, SQ], bf, tag="xbf")       # x own half, scale 1
            q8 = big.tile([P, NC8, SQ], f8, tag="q8")         # q * S_Q/sqrt(HD), feat-major
            k8 = big.tile([P, NC8, S], f8, tag="k8")          # k * S_K
            v8 = big.tile([P, NKT, H], f8, tag="v8")          # v * S_V, token-major
            ctx8 = big.tile([P, NC8, SQ], f8, tag="ctx8")     # ctx * S_CTX, feat-major
            attbf = big.tile([P, NC8, SQ], bf, tag="attbf")   # attended, scale 1

            # ---------- phase X ----------
            # host sends h,p pre-scaled by S_X; xbf holds 16x (w1's x-half is
            # divided by 16 on host to compensate). Other half: add -> fp8
            # directly; own half: add -> bf16, then ACT copy -> fp8.
            with tc.tile_pool(name="tmpx", bufs=2) as tmp:
                for c4 in range(NC8 // 2):
                    ht4 = tmp.tile([P, 2, S], bf, tag="ht4")
                    pt4 = tmp.tile([P, 2, S], bf, tag="pt4")
                    nc.sync.dma_start(
                        ht4[:], hT[c4 * 256:(c4 + 1) * 256, :]
                        .rearrange("(c p) s -> p c s", p=P))
                    nc.sync.dma_start(
                        pt4[:], pT[c4 * 256:(c4 + 1) * 256, :]
                        .rearrange("(c p) s -> p c s", p=P))
                    for i in range(2):
                        c8 = 2 * c4 + i
                        nc.vector.tensor_tensor(out=xbf[:, c8, :],
                                                in0=ht4[:, i, 0:SQ],
                                                in1=pt4[:, i, 0:SQ], op=OP.add)
                        nc.scalar.activation(x8[:, c8, 0:SQ], xbf[:, c8, :],
                                             ACT.Identity, bias=0.0, scale=1.0)
                        nc.vector.tensor_tensor(out=x8[:, c8, SQ:S],
                                                in0=ht4[:, i, SQ:S],
                                                in1=pt4[:, i, SQ:S], op=OP.add)

            # weights
            wq_sb = cload([P, NC8, H], "wq", wq8, f8)
            wk_sb = cload([P, NC8, H], "wk", wk8, f8)
            wv_sb = cload([P, NC8, H], "wv", wv8, f8)
            ow_sb = cload([P, NC8, H], "ow", ow8, f8)
            w1_sb = cload([P, 2 * NC8, MD], "w1", w1bf, bf)
            w2_sb = cload([P, MD // P, MD2], "w2", w2bf, bf)

            # ---------- phase QKV (pair-merged PSUM tiles, 1024-wide stagings) ----------
            with tc.tile_pool(name="ps_mm1", bufs=3, space="PSUM") as ps1:
                # Q/K only for head 0 here; heads 1..7 are emitted inside
                # the attention loop (emit_qk) so exp can start ~40us earlier.
                def emit_qk_with(pool, tag, h):
                    psq = pool.tile([P, 2, 512], f32, tag=tag, name="psq")
                    for j in range(NC8 // 2):
                        for qb in range(NQB):
                            nc.tensor.matmul(
                                psq[:, qb, :],
                                lhsT=wq_sb[:, 2 * j:2 * j + 2, h * P:(h + 1) * P],
                                rhs=x8[:, 2 * j:2 * j + 2, qb * 512:(qb + 1) * 512],
                                start=(j == 0), stop=(j == NC8 // 2 - 1),
                                perf_mode=DR)
                    nc.scalar.activation(
                        q8[:, h, :], psq[:], ACT.Identity,
                        bias=bq_sb[:, h:h + 1],
                        scale=S_Q / (np.sqrt(HD) * S_X * S_W))
                    for sp in range(2):
                        psk = pool.tile([P, 2, 512], f32, tag=tag, name="psk")
                        for j in range(NC8 // 2):
                            for i in range(2):
                                sb = 2 * sp + i
                                nc.tensor.matmul(
                                    psk[:, i, :],
                                    lhsT=wk_sb[:, 2 * j:2 * j + 2, h * P:(h + 1) * P],
                                    rhs=x8[:, 2 * j:2 * j + 2, sb * 512:(sb + 1) * 512],
                                    start=(j == 0), stop=(j == NC8 // 2 - 1),
                                    perf_mode=DR)
                        nc.vector.tensor_scalar(
                            k8[:, h, sp * 1024:(sp + 1) * 1024], psk[:],
                            S_K / (S_X * S_W), None, op0=OP.mult)

                for dt in range(NC8):
                    emit_qk_with(ps1, "mmp", dt)
                # V token-major [tok, feat], no bias (folded into b1_eff)
                for tt in range(NKT):
                    psv = ps1.tile([P, 2, 512], f32, tag="mmp", name="psv")
                    for j in range(NC8 // 2):
                        for db in range(2):
                            nc.tensor.matmul(
                                psv[:, db, :],
                                lhsT=x8[:, 2 * j:2 * j + 2, tt * P:(tt + 1) * P],
                                rhs=wv_sb[:, 2 * j:2 * j + 2, db * 512:(db + 1) * 512],
                                start=(j == 0), stop=(j == NC8 // 2 - 1),
                                perf_mode=DR)
                    nc.vector.tensor_scalar(
                        v8[:, tt, :], psv[:], S_V / (S_X * S_W), None, op0=OP.mult)

            # ---------- attention + out-proj + w1, interleaved ----------
            # qb-outer group order; once a qb's 8 head-groups are done, its
            # out-proj and w1 matmuls become PE filler work emitted between
            # the next groups (the ACT engine is the bottleneck during exp).
            NFT = MD // P      # 2 feature tiles of h1
            with tc.tile_pool(name="exs", bufs=2) as exs, \
                 tc.tile_pool(name="asml", bufs=2) as asml, \
                 tc.tile_pool(name="mw", bufs=1) as mw, \
                 tc.tile_pool(name="msml", bufs=1) as sml, \
                 tc.tile_pool(name="ps_sc", bufs=2, space="PSUM") as ps_sc, \
                 tc.tile_pool(name="ps_ctx", bufs=1, space="PSUM") as ps_ctx, \
                 tc.tile_pool(name="ps_dn", bufs=1, space="PSUM") as ps_dn, \
                 tc.tile_pool(name="ps_io", bufs=2, space="PSUM") as ps_io:
                h1p = mw.tile([P, NFT, SQ], bf, tag="h1p")
                h1sq = mw.tile([P, NFT, SQ], bf, tag="h1sq")
                h1n = mw.tile([P, NFT, SQ], bf, tag="h1n")
                stat = mw.tile([P, 3, SQ], f32, tag="stat")
                res_sb = mw.tile([P, NTT], f32, tag="res")
                nmean, work, m2r = stat[:, 0, :], stat[:, 1, :], stat[:, 2, :]
                ex2m = varm = rstd = work

                def emit_oproj(qb, dt):
                    qsl = slice(qb * 512, (qb + 1) * 512)
                    pso = ps_io.tile([P, 512], f32, tag="psio", name="pso")
                    for j in range(NC8 // 2):
                        nc.tensor.matmul(
                            pso[:],
                            lhsT=ow_sb[:, 2 * j:2 * j + 2, dt * P:(dt + 1) * P],
                            rhs=ctx8[:, 2 * j:2 * j + 2, qsl],
                            start=(j == 0), stop=(j == NC8 // 2 - 1),
                            perf_mode=DR)
                    # bias (incl. folded V bias) lives in b1_eff on host
                    nc.vector.tensor_scalar(
                        attbf[:, dt, qsl], pso[:],
                        1.0 / (S_CTX * S_W), None, op0=OP.mult)

                def emit_w1(qb, ft):
                    qsl = slice(qb * 512, (qb + 1) * 512)
                    psf = ps_io.tile([P, 512], f32, tag="psio", name="psf")
                    for c16 in range(2 * NC8):
                        if c16 < NC8:
                            rhs = xbf[:, c16, qsl]
                        else:
                            rhs = attbf[:, c16 - NC8, qsl]
                        nc.tensor.matmul(
                            psf[:],
                            lhsT=w1_sb[:, c16, ft * P:(ft + 1) * P],
                            rhs=rhs,
                            start=(c16 == 0), stop=(c16 == 2 * NC8 - 1))
                    nc.scalar.activation(
                        h1p[:, ft, qsl], psf[:],
                        ACT.Identity, bias=b1_sb[:, ft:ft + 1], scale=1.0)

                hb2_all = mw.tile([P, NTT, MD2], f32, tag="hb2_all")

                def emit_ln1(qb):
                    qsl = slice(qb * 512, (qb + 1) * 512)
                    for ft in range(NFT):
                        nc.vector.tensor_tensor(out=h1sq[:, ft, qsl],
                                                in0=h1p[:, ft, qsl],
                                                in1=h1p[:, ft, qsl], op=OP.mult)
                    psA = ps_io.tile([P, 512], f32, tag="psio", name="psA")
                    psB = ps_io.tile([P, 512], f32, tag="psio", name="psB")
                    for ft in range(NFT):
                        nc.tensor.matmul(psA[:], lhsT=onesbf[:],
                                         rhs=h1p[:, ft, qsl],
                                         start=(ft == 0), stop=(ft == NFT - 1))
                    for ft in range(NFT):
                        nc.tensor.matmul(psB[:], lhsT=onesbf[:],
                                         rhs=h1sq[:, ft, qsl],
                                         start=(ft == 0), stop=(ft == NFT - 1))
                    nc.vector.tensor_scalar_mul(nmean[:, qsl], psA[:], -1.0 / MD)
                    nc.vector.tensor_scalar_mul(ex2m[:, qsl], psB[:], 1.0 / MD)
                    nc.vector.tensor_tensor(out=m2r[:, qsl], in0=nmean[:, qsl],
                                            in1=nmean[:, qsl], op=OP.mult)
                    nc.vector.tensor_tensor(out=work[:, qsl], in0=work[:, qsl],
                                            in1=m2r[:, qsl], op=OP.subtract)
                    # rstd = exp(-0.5 * ln(var + eps))
                    nc.scalar.activation(varm[:, qsl], varm[:, qsl], ACT.Ln,
                                         bias=eps_sb[:, 0:1], scale=1.0)
                    nc.scalar.activation(rstd[:, qsl], varm[:, qsl], ACT.Exp,
                                         bias=0.0, scale=-0.5)
                    for ft in range(NFT):
                        nc.vector.tensor_tensor(out=h1n[:, ft, qsl],
                                                in0=h1p[:, ft, qsl],
                                                in1=nmean[:, qsl], op=OP.add)
                        nc.vector.tensor_tensor(out=h1n[:, ft, qsl],
                                                in0=h1n[:, ft, qsl],
                                                in1=rstd[:, qsl], op=OP.mult)
                        nc.scalar.activation(h1n[:, ft, qsl], h1n[:, ft, qsl],
                                             ACT.Relu, bias=be1_sb[:, ft:ft + 1],
                                             scale=g1_sb[:, ft:ft + 1])

                def emit_h2(tt, _unused=None):
                    ph2_t = ps_io.tile([P, 512], f32, tag="psio", name="ph2")
                    ph2 = ph2_t[:, :MD2]
                    for ft in range(MD // P):
                        nc.tensor.matmul(ph2, lhsT=h1n[:, ft, tt * P:(tt + 1) * P],
                                         rhs=w2_sb[:, ft, :],
                                         start=(ft == 0), stop=(ft == MD // P - 1))
                    nc.vector.scalar_tensor_tensor(out=hb2_all[:, tt, :], in0=ph2,
                                                   scalar=1.0, in1=b2_sb[:],
                                                   op0=OP.mult, op1=OP.add)

                F2 = float(MD2)

                def emit_tail(qb, _unused=None):
                    r = slice(qb * NTT // NQB, (qb + 1) * NTT // NQB)
                    nt = NTT // NQB
                    hb2 = hb2_all[:, r, :]
                    sums2 = sml.tile([P, NTT], f32, tag="sums2")
                    nc.vector.reduce_sum(sums2[:, r], hb2,
                                         axis=mybir.AxisListType.X)
                    msq = sml.tile([P, NTT, MD2], f32, tag="msq")
                    ssq2 = sml.tile([P, NTT], f32, tag="ssq2")
                    nc.vector.tensor_tensor(out=msq[:, r, :], in0=hb2,
                                            in1=hb2, op=OP.mult)
                    nc.vector.reduce_sum(ssq2[:, r], msq[:, r, :],
                                         axis=mybir.AxisListType.X)
                    nm2 = sml.tile([P, NTT], f32, tag="nm2")
                    nc.vector.tensor_scalar_mul(nm2[:, r], sums2[:, r], -1.0 / F2)
                    ex22 = sml.tile([P, NTT], f32, tag="ex22")
                    nc.vector.tensor_scalar_mul(ex22[:, r], ssq2[:, r], 1.0 / F2)
                    mm2 = sml.tile([P, NTT], f32, tag="mm2")
                    nc.vector.tensor_tensor(out=mm2[:, r], in0=nm2[:, r],
                                            in1=nm2[:, r], op=OP.mult)
                    var2 = sml.tile([P, NTT], f32, tag="var2")
                    nc.vector.tensor_tensor(out=var2[:, r], in0=ex22[:, r],
                                            in1=mm2[:, r], op=OP.subtract)
                    std2 = sml.tile([P, NTT], f32, tag="std2")
                    nc.scalar.activation(std2[:, r], var2[:, r], ACT.Sqrt,
                                         bias=eps_sb[:, 0:1], scale=1.0)
                    rstd2 = sml.tile([P, NTT], f32, tag="rstd2")
                    nc.vector.reciprocal_approx_fast(rstd2[:, r], std2[:, r])
                    t1a = sml.tile([P, NTT, MD2], f32, tag="t1a")
                    nc.vector.tensor_tensor(
                        out=t1a[:, r, :], in0=hb2,
                        in1=nm2[:, r, None].to_broadcast([P, nt, MD2]), op=OP.add)
                    nc.vector.tensor_tensor(
                        out=t1a[:, r, :], in0=t1a[:, r, :],
                        in1=rstd2[:, r, None].to_broadcast([P, nt, MD2]),
                        op=OP.mult)
                    nc.vector.tensor_tensor(
                        out=t1a[:, r, :], in0=t1a[:, r, :],
                        in1=g2_sb[:, None, :].to_broadcast([P, nt, MD2]),
                        op=OP.mult)
                    nc.vector.tensor_tensor(
                        out=t1a[:, r, :], in0=t1a[:, r, :],
                        in1=be2_sb[:, None, :].to_broadcast([P, nt, MD2]),
                        op=OP.add)
                    nc.vector.tensor_scalar_max(t1a[:, r, :], t1a[:, r, :], 0.0)
                    nc.vector.tensor_tensor(
                        out=t1a[:, r, :], in0=t1a[:, r, :],
                        in1=w3_sb[:, None, :].to_broadcast([P, nt, MD2]),
                        op=OP.mult)
                    base8 = sml.tile([P, NTT], f32, tag="base8")
                    nc.vector.reduce_sum(base8[:, r], t1a[:, r, :],
                                         axis=mybir.AxisListType.X)
                    nc.vector.tensor_tensor(
                        out=base8[:, r], in0=base8[:, r],
                        in1=b3_sb[:, 0:1].to_broadcast([P, nt]), op=OP.add)
                    imp1a = sml.tile([P, NTT], f32, tag="imp1a")
                    nc.vector.tensor_scalar_add(imp1a[:, r], imp_all[:, r], 1.0)
                    nc.vector.tensor_tensor(out=base8[:, r], in0=base8[:, r],
                                            in1=imp1a[:, r], op=OP.mult)
                    nc.vector.tensor_scalar(base8[:, r], base8[:, r], MAX_W, MIN_W,
                                            op0=OP.min, op1=OP.max)
                    nc.vector.tensor_tensor(out=res_sb[:, r], in0=base8[:, r],
                                            in1=maskf_sb[:, r], op=OP.mult)
                    nc.sync.dma_start(
                        out[:].rearrange("(t p) -> p t", p=P)[:, r], res_sb[:, r])

                groups = [(h, qb) for qb in range(NQB) for h in range(NH)]
                NPAIR = NKT // 2
                fillers = []

                def emit_scexp(g, ex8):
                    h, qb = groups[g]
                    qsl = slice(qb * 512, (qb + 1) * 512)
                    for t in range(NPAIR):
                        scp = ps_sc.tile([P, 2, 512], f32, tag="scp")
                        for i in range(2):
                            nc.tensor.matmul(scp[:, i, :],
                                             lhsT=k8[:, h, (2 * t + i) * P:
                                                  (2 * t + i + 1) * P],
                                             rhs=q8[:, h, qsl],
                                             start=True, stop=True)
                        if 2 * t < nex_act:
                            nc.scalar.activation(ex8[:, 2 * t:2 * t + 2, :], scp[:],
                                                 ACT.Exp,
                                                 bias=kb_sb[:, 2 * t:2 * t + 1],
                                                 scale=1.0 / SCORE_PS)
                        else:
                            # Schraudolph exp straight to e4m3: uint8 convert
                            # saturates negatives to 0 (== zero attn weight)
                            nc.vector.tensor_scalar(
                                ex8[:, 2 * t:2 * t + 2, :].bitcast(u8),
                                scp[:], 8.0 * LOG2E / SCORE_PS,
                                kbs_sb[:, 2 * t:2 * t + 1],
                                op0=OP.mult, op1=OP.add)

                ex_tiles = {}

                def alloc_ex():
                    ex8 = exs.tile([P, NKT, 512], f8, tag="ex8", name="ex8")
                    return (ex8,)

                ex_tiles[0] = alloc_ex()
                emit_scexp(0, *ex_tiles[0])
                for g, (h, qb) in enumerate(groups):
                    qsl = slice(qb * 512, (qb + 1) * 512)
                    ex8, = ex_tiles.pop(g)
                    if g + 1 < len(groups):
                        ex_tiles[g + 1] = alloc_ex()
                        emit_scexp(g + 1, *ex_tiles[g + 1])
                    cps = ps_ctx.tile([P, 512], f32, tag="cps")
                    dn = ps_dn.tile([P, 512], f32, tag="dn")
                    n_mm = NKT // 2
                    for t in range(n_mm):
                        nc.tensor.matmul(cps[:],
                                         lhsT=v8[:, 2 * t:2 * t + 2, h * P:(h + 1) * P],
                                         rhs=ex8[:, 2 * t:2 * t + 2, :],
                                         start=(t == 0), stop=(t == n_mm - 1),
                                         perf_mode=DR)
                        nc.tensor.matmul(dn[:], lhsT=ones8[:],
                                         rhs=ex8[:, 2 * t:2 * t + 2, :],
                                         start=(t == 0), stop=(t == n_mm - 1),
                                         perf_mode=DR)
                    rcb = asml.tile([P, 512], f32, tag="rcb")
                    nc.vector.reciprocal_approx_fast(rcb[:], dn[:])
                    nc.vector.scalar_tensor_tensor(
                        out=ctx8[:, h, qsl], in0=cps[:], scalar=S_CTX / S_V,
                        in1=rcb[:], op0=OP.mult, op1=OP.mult)
                    if h == NH - 1:
                        fillers += [(emit_oproj, qb, dt) for dt in range(NC8)]
                        fillers += [(emit_w1, qb, ft) for ft in range(NFT)]
                        fillers += [(lambda q, _u: emit_ln1(q), qb, None)]
                        fillers += [(emit_h2, tt, None)
                                    for tt in range(qb * NTT // NQB,
                                                    (qb + 1) * NTT // NQB)]
                        fillers += [(emit_tail, qb, None)]
                    for _ in range(2):
                        if fillers:
                            fn, a, b2_ = fillers.pop(0)
                            fn(a, b2_)
                while fillers:
                    fn, a, b2_ = fillers.pop(0)
                    fn(a, b2_)

                # LN2/final emitted per-qb as fillers (emit_tail)

    nc.compile()
    return nc


def _get_program():
    nex_act = int(os.environ.get("KB_NEX_ACT", "14"))
    key = ("nc", nex_act)
    if key not in _CACHE:
        _CACHE[key] = _build(nex_act)
    return _CACHE[key]


def _prep_in_maps(inputs):
    import ml_dtypes
    bf16 = ml_dtypes.bfloat16
    f8 = ml_dtypes.float8_e4m3

    hidden = np.asarray(inputs["hidden_states"], dtype=np.float32)
    token_ids = np.asarray(inputs["token_ids"], dtype=np.int32)
    mask = np.asarray(inputs["attention_mask"]).astype(bool)
    pos = np.asarray(inputs["pos_embed"], dtype=np.float32)
    in_proj_w = np.asarray(inputs["in_proj_w"], dtype=np.float32)
    in_proj_b = np.asarray(inputs["in_proj_b"], dtype=np.float32)
    out_w = np.asarray(inputs["out_w"], dtype=np.float32)
    out_b = np.asarray(inputs["out_b"], dtype=np.float32)
    w1 = np.asarray(inputs["w1"], dtype=np.float32)
    b1 = np.asarray(inputs["b1"], dtype=np.float32)
    g1 = np.asarray(inputs["g1"], dtype=np.float32)
    beta1 = np.asarray(inputs["beta1"], dtype=np.float32)
    w2 = np.asarray(inputs["w2"], dtype=np.float32)
    b2 = np.asarray(inputs["b2"], dtype=np.float32)
    g2 = np.asarray(inputs["g2"], dtype=np.float32)
    beta2 = np.asarray(inputs["beta2"], dtype=np.float32)
    w3 = np.asarray(inputs["w3"], dtype=np.float32)
    b3 = np.asarray(inputs["b3"], dtype=np.float32)
    table = np.asarray(inputs["importance_table"], dtype=np.float32)

    B, S_, H_ = hidden.shape
    assert (B, S_, H_) == (4, S, H), (B, S_, H_)

    posT = np.ascontiguousarray(pos[0].T)                      # [H, S]
    wqT = in_proj_w[0:H].T
    wkT = in_proj_w[H:2 * H].T
    wvT = in_proj_w[2 * H:3 * H].T
    bq = in_proj_b[0:H]
    bk = in_proj_b[H:2 * H]
    bv = in_proj_b[2 * H:3 * H]
    owT = out_w.T
    # softmax weights sum to 1, so the V bias passes through attention as a
    # constant: fold it (and out_b) into the out-proj bias, then fold that
    # into the w1 bias (b1_eff), since attended only feeds w1.
    ob_eff = bv @ out_w.T + out_b
    b1_eff = b1 + w1[:, H:2 * H] @ ob_eff

    def warr(wT, s=S_W):   # [H, N] -> [128, 8, N] fp8 scaled
        return np.ascontiguousarray(
            np.clip(wT * s, -224.0, 224.0)
            .reshape(NC8, P, -1).transpose(1, 0, 2)).astype(f8)

    def cmaj(v, s=1.0):   # [F] -> [128, F/128]
        return np.ascontiguousarray((v * s).reshape(-1, P).T.astype(np.float32))

    def bcast(v):  # [F] -> [128, F]
        return np.ascontiguousarray(
            np.broadcast_to(v[None, :], (P, v.shape[0])).astype(np.float32))

    shared = {
        "wq8": warr(wqT), "wk8": warr(wkT),
        "wv8": warr(wvT), "ow8": warr(owT),
        # x is stored on-chip as 16x (bf16); compensate in w1's x-half
        "w1bf": np.ascontiguousarray(
            np.concatenate([w1.T[:H] / S_X, w1.T[H:]], axis=0)
            .reshape(2 * NC8, P, MD).transpose(1, 0, 2)).astype(bf16),
        "w2bf": np.ascontiguousarray(
            w2.T.reshape(MD // P, P, MD2).transpose(1, 0, 2)).astype(bf16),
        "bq_c": cmaj(bq, S_Q / np.sqrt(HD)),
        "b1_c": cmaj(b1_eff), "g1_c": cmaj(g1), "be1_c": cmaj(beta1),
        "b2_b": bcast(b2), "g2_b": bcast(g2), "be2_b": bcast(beta2),
        "w3_b": bcast(w3[0]), "b3_c": np.full((P, 1), b3[0], dtype=np.float32),
        "table": np.ascontiguousarray(table[:, None]),
    }
    in_maps = []
    for c in range(8):
        b = c // 2
        half = c % 2
        own = slice(half * SQ, (half + 1) * SQ)
        oth = slice((1 - half) * SQ, (2 - half) * SQ)
        hT_b = hidden[b].T * S_X
        posT_s = posT * S_X
        hT_arr = np.ascontiguousarray(
            np.concatenate([hT_b[:, own], hT_b[:, oth]], axis=1)).astype(bf16)
        pT_arr = np.ascontiguousarray(
            np.concatenate([posT_s[:, own], posT_s[:, oth]], axis=1)).astype(bf16)
        kb = np.where(mask[b], 0.0, -1e9).astype(np.float32)
        kb_arr = np.concatenate([kb[own], kb[oth]]) - C_SHIFT
        kbs_arr = (56.0 - 0.5) + 8.0 * LOG2E * (
            np.concatenate([kb[own], kb[oth]]) - C_SHIFT)
        m = {
            "hT": hT_arr, "pT": pT_arr,
            "kb_c": np.ascontiguousarray(kb_arr.reshape(-1, P).T),
            "kbs_c": np.ascontiguousarray(
                kbs_arr.reshape(-1, P).T.astype(np.float32)),
            "maskf": np.ascontiguousarray(
                mask[b, own].astype(np.float32).reshape(-1, P).T),
            "tok": np.ascontiguousarray(token_ids[b, own][:, None]),
        }
        m.update(shared)
        in_maps.append(m)
    return in_maps


def _assemble(res):
    full = np.zeros((4, S), dtype=np.float32)
    for c in range(8):
        b = c // 2
        half = c % 2
        full[b, half * SQ:(half + 1) * SQ] = res.results[c]["out"]
    return full


def kernel(**inputs) -> np.ndarray:
    from concourse.bass_utils import run_bass_kernel_spmd
    in_maps = _prep_in_maps(inputs)
    nc = _get_program()
    res = run_bass_kernel_spmd(nc, in_maps, list(range(8)))
    return _assemble(res)


def run_traced(inputs, **kwargs):
    from concourse.bass_utils import run_bass_kernel_spmd
    in_maps = _prep_in_maps(inputs)
    nc = _get_program()
    return run_bass_kernel_spmd(nc, in_maps, list(range(8)), trace=True, **kwargs)


# revision 44
# speedup vs baseline: 1.0350x; 1.0062x over previous
"""Trainium2 Bass kernel for EnhancedMetaWeightNetwork (v2: fp8/bf16).

Full (unsharded) inputs in, full output out. 8 NeuronCores: core c handles
batch b = c // 2 and query-row half c % 2 (SQ=1024 own queries, full S=2048
keys; K/V recomputed per core pair — no collectives).

v2 design (vs fp32r v1):
  - attention path in fp8 e4m3 (QKV/V/AV/dn/out-proj use DoubleRow matmuls,
    256-deep contraction per instruction); scores matmul plain fp8.
  - meta-MLP in bf16 (fp8 there fails the accuracy budget); x stored as 16x
    bf16 with w1's x-half pre-divided by 16 on host.
  - pair-merged exp: scores land in 2-bank [P,2,512] PSUM tiles, one 1024-wide
    exp instruction per pair. exp(score - C_SHIFT) on ACT -> fp8 ex (C_SHIFT
    keeps exp < e4m3 max 240; cancels exactly in softmax). The last
    NKT-KB_NEX_ACT key-tiles per group instead use a Schraudolph exp on DVE
    writing e4m3 bytes directly via uint8 convert (negatives saturate to 0 ==
    zero attention weight), so AV/dn stay DoubleRow everywhere.
  - biases folded on host: V-bias + out_b -> b1_eff (softmax weights sum to
    1); K-bias dropped (adds a per-query constant to scores -> softmax
    cancels it).
  - qb-outer attention groups; once a qb finishes, its out-proj, w1, LN1, h2
    and LN2-final become PE/DVE filler work interleaved into the next qb's
    exp-bound window.
  - PSUM->SBUF stagings split ACT (needs bias: Q, h1p) / DVE (K, V, O);
    softmax normalize + reciprocal_approx_fast on DVE; everything resident in
    SBUF, no DRAM spills.
"""

import os
import numpy as np

H = 1024
NH = 8
HD = 128           # head dim
S = 2048           # keys / full sequence
SQ = 1024          # own query rows per core
MD = 256
MD2 = 128
VOCAB = 32000
MIN_W, MAX_W = 0.1, 5.0
LN_EPS = 1e-5
P = 128
NKT = S // P       # 16 key tiles
NC8 = H // P       # 8 feature chunks
NTT = SQ // P      # 8 own token tiles
NQB = SQ // 512    # 2 query blocks

# fp8 scales
S_X = 16.0
S_W = 1024.0       # fp8 scale for wq/wk/wv/ow (values ~N(0, 0.02^2*1024) -> amax ~0.11)
S_Q = 64.0         # includes 1/sqrt(HD)
S_K = 16.0
S_V = 32.0
S_CTX = 32.0
C_SHIFT = 2.0
SCORE_PS = S_Q * S_K          # score_true * SCORE_PS in PSUM

# Schraudolph bf16 exp constants: bf16 = bitcast(int16(x*128*log2e + (16256-c)))
LOG2E = 1.4426950408889634
SCHR_C = 9.0

_CACHE = {}


def _build(nex_act):
    """nex_act: number of key-tiles per (h,qb) exp'd on ACT (fp8, even);
    the remaining NKT-nex_act use DVE Schraudolph (bf16)."""
    import concourse.bass as bass
    import concourse.mybir as mybir
    import concourse.tile as tile
    from concourse import bacc

    f32 = mybir.dt.float32
    f8 = mybir.dt.float8e4
    bf = mybir.dt.bfloat16
    i16 = mybir.dt.int16
    u8 = mybir.dt.uint8
    i32 = mybir.dt.int32
    OP = mybir.AluOpType
    ACT = mybir.ActivationFunctionType
    DR = mybir.MatmulPerfMode.DoubleRow

    nex_dve = NKT - nex_act
    assert nex_act % 2 == 0 and nex_dve % 2 == 0

    nc = bacc.Bacc("TRN2", target_bir_lowering=False, debug=False,
                   enable_asserts=False, num_devices=8)

    # ---------------- DRAM parameters ----------------
    dp = nc.declare_dram_parameter
    hT = dp("hT", [H, S], bf, isOutput=False)             # hidden[b].T * S_X, own half first
    pT = dp("pT", [H, S], bf, isOutput=False)             # pos.T * S_X
    wq8 = dp("wq8", [P, NC8, H], f8, isOutput=False)      # * s_wq, (c p) n -> p c n
    wk8 = dp("wk8", [P, NC8, H], f8, isOutput=False)
    wv8 = dp("wv8", [P, NC8, H], f8, isOutput=False)
    ow8 = dp("ow8", [P, NC8, H], f8, isOutput=False)
    w1bf = dp("w1bf", [P, 2 * NC8, MD], bf, isOutput=False)
    w2bf = dp("w2bf", [P, MD // P, MD2], bf, isOutput=False)
    bq_c = dp("bq_c", [P, NC8], f32, isOutput=False)      # bq * S_Q/sqrt(HD)
    b1_c = dp("b1_c", [P, MD // P], f32, isOutput=False)  # b1 + w1[:, H:] @ ob_eff
    g1_c = dp("g1_c", [P, MD // P], f32, isOutput=False)
    be1_c = dp("be1_c", [P, MD // P], f32, isOutput=False)
    b2_b = dp("b2_b", [P, MD2], f32, isOutput=False)
    g2_b = dp("g2_b", [P, MD2], f32, isOutput=False)
    be2_b = dp("be2_b", [P, MD2], f32, isOutput=False)
    w3_b = dp("w3_b", [P, MD2], f32, isOutput=False)
    b3_c = dp("b3_c", [P, 1], f32, isOutput=False)
    kb_c = dp("kb_c", [P, NKT], f32, isOutput=False)      # mask bias - C_SHIFT (exp bias)
    kbs_c = dp("kbs_c", [P, NKT], f32, isOutput=False)    # schraudolph additive const
    maskf = dp("maskf", [P, NTT], f32, isOutput=False)
    tok = dp("tok", [SQ, 1], i32, isOutput=False)
    table = dp("table", [VOCAB, 1], f32, isOutput=False)
    out = dp("out", [SQ], f32, isOutput=True)

    with tile.TileContext(nc) as tc:
        with tc.tile_pool(name="const", bufs=1) as cst, \
             tc.tile_pool(name="big", bufs=1) as big:

            # ---------------- constants ----------------
            ones8 = cst.tile([P, 2, P], f8, tag="ones8")
            nc.any.memset(ones8[:], 1.0)
            onesbf = cst.tile([P, P], bf, tag="onesbf")
            nc.any.memset(onesbf[:], 1.0)
            eps_sb = cst.tile([P, 1], f32, tag="eps")
            nc.any.memset(eps_sb[:], LN_EPS)

            def cload(shape, tag, src, dt=f32):
                t = cst.tile(shape, dt, tag=tag)
                nc.sync.dma_start(t[:], src[:])
                return t

            kb_sb = cload([P, NKT], "kb", kb_c)
            kbs_sb = cload([P, NKT], "kbs", kbs_c)
            maskf_sb = cload([P, NTT], "maskf", maskf)
            b3_sb = cload([P, 1], "b3", b3_c)
            w3_sb = cload([P, MD2], "w3", w3_b)
            bq_sb = cload([P, NC8], "bq", bq_c)
            b1_sb = cload([P, MD // P], "b1c", b1_c)
            g1_sb = cload([P, MD // P], "g1c", g1_c)
            be1_sb = cload([P, MD // P], "be1c", be1_c)
            b2_sb = cload([P, MD2], "b2", b2_b)
            g2_sb = cload([P, MD2], "g2", g2_b)
            be2_sb = cload([P, MD2], "be2", be2_b)

            imp_all = cst.tile([P, NTT], f32, tag="imp_all")
            itt_all = cst.tile([P, NTT], i32, tag="itt_all")
            nc.sync.dma_start(itt_all[:],
                              tok[:, 0].rearrange("(t p) -> p t", p=P))
            for tt in range(NTT):
                nc.gpsimd.indirect_dma_start(
                    out=imp_all[:, tt:tt + 1], out_offset=None, in_=table[:],
                    in_offset=bass.IndirectOffsetOnAxis(ap=itt_all[:, tt:tt + 1],
                                                        axis=0))

            # persistent activations
            x8 = big.tile([P, NC8, S], f8, tag="x8")          # x * S_X
            xbf = big.tile([P, NC8, SQ], bf, tag="xbf")       # x own half, scale 1
            q8 = big.tile([P, NC8, SQ], f8, tag="q8")         # q * S_Q/sqrt(HD), feat-major
            k8 = big.tile([P, NC8, S], f8, tag="k8")          # k * S_K
            v8 = big.tile([P, NKT, H], f8, tag="v8")          # v * S_V, token-major
            ctx8 = big.tile([P, NC8, SQ], f8, tag="ctx8")     # ctx * S_CTX, feat-major
            attbf = big.tile([P, NC8, SQ], bf, tag="attbf")   # attended, scale 1

            # ---------- phase X ----------
            # host sends h,p pre-scaled by S_X; xbf holds 16x (w1's x-half is
            # divided by 16 on host to compensate). Other half: add -> fp8
            # directly; own half: add -> bf16, then ACT copy -> fp8.
            with tc.tile_pool(name="tmpx", bufs=2) as tmp:
                for c4 in range(NC8 // 2):
                    ht4 = tmp.tile([P, 2, S], bf, tag="ht4")
                    pt4 = tmp.tile([P, 2, S], bf, tag="pt4")
                    nc.sync.dma_start(
                        ht4[:], hT[c4 * 256:(c4 + 1) * 256, :]
                        .rearrange("(c p) s -> p c s", p=P))
                    nc.sync.dma_start(
                        pt4[:], pT[c4 * 256:(c4 + 1) * 256, :]
                        .rearrange("(c p) s -> p c s", p=P))
                    for i in range(2):
                        c8 = 2 * c4 + i
                        nc.vector.tensor_tensor(out=xbf[:, c8, :],
                                                in0=ht4[:, i, 0:SQ],
                                                in1=pt4[:, i, 0:SQ], op=OP.add)
                        nc.scalar.activation(x8[:, c8, 0:SQ], xbf[:, c8, :],
                                             ACT.Identity, bias=0.0, scale=1.0)
                        nc.vector.tensor_tensor(out=x8[:, c8, SQ:S],
                                                in0=ht4[:, i, SQ:S],
                                                in1=pt4[:, i, SQ:S], op=OP.add)

            # weights
            wq_sb = cload([P, NC8, H], "wq", wq8, f8)
            wk_sb = cload([P, NC8, H], "wk", wk8, f8)
            wv_sb = cload([P, NC8, H], "wv", wv8, f8)
            ow_sb = cload([P, NC8, H], "ow", ow8, f8)
            w1_sb = cload([P, 2 * NC8, MD], "w1", w1bf, bf)
            w2_sb = cload([P, MD // P, MD2], "w2", w2bf, bf)

            # ---------- phase QKV (pair-merged PSUM tiles, 1024-wide stagings) ----------
            with tc.tile_pool(name="ps_mm1", bufs=3, space="PSUM") as ps1:
                # Q/K only for head 0 here; heads 1..7 are emitted inside
                # the attention loop (emit_qk) so exp can start ~40us earlier.
                def emit_qk_with(pool, tag, h):
                    psq = pool.tile([P, 2, 512], f32, tag=tag, name="psq")
                    for j in range(NC8 // 2):
                        for qb in range(NQB):
                            nc.tensor.matmul(
                                psq[:, qb, :],
                                lhsT=wq_sb[:, 2 * j:2 * j + 2, h * P:(h + 1) * P],
                                rhs=x8[:, 2 * j:2 * j + 2, qb * 512:(qb + 1) * 512],
                                start=(j == 0), stop=(j == NC8 // 2 - 1),
                                perf_mode=DR)
                    nc.scalar.activation(
                        q8[:, h, :], psq[:], ACT.Identity,
                        bias=bq_sb[:, h:h + 1],
                        scale=S_Q / (np.sqrt(HD) * S_X * S_W))
                    for sp in range(2):
                        psk = pool.tile([P, 2, 512], f32, tag=tag, name="psk")
                        for j in range(NC8 // 2):
                            for i in range(2):
                                sb = 2 * sp + i
                                nc.tensor.matmul(
                                    psk[:, i, :],
                                    lhsT=wk_sb[:, 2 * j:2 * j + 2, h * P:(h + 1) * P],
                                    rhs=x8[:, 2 * j:2 * j + 2, sb * 512:(sb + 1) * 512],
                                    start=(j == 0), stop=(j == NC8 // 2 - 1),
                                    perf_mode=DR)
                        nc.vector.tensor_scalar(
                            k8[:, h, sp * 1024:(sp + 1) * 1024], psk[:],
                            S_K / (S_X * S_W), None, op0=OP.mult)

                for dt in range(NC8):
                    emit_qk_with(ps1, "mmp", dt)
                # V token-major [tok, feat], no bias (folded into b1_eff)
                for tt in range(NKT):
                    psv = ps1.tile([P, 2, 512], f32, tag="mmp", name="psv")
                    for j in range(NC8 // 2):
                        for db in range(2):
                            nc.tensor.matmul(
                                psv[:, db, :],
                                lhsT=x8[:, 2 * j:2 * j + 2, tt * P:(tt + 1) * P],
                                rhs=wv_sb[:, 2 * j:2 * j + 2, db * 512:(db + 1) * 512],
                                start=(j == 0), stop=(j == NC8 // 2 - 1),
                                perf_mode=DR)
                    nc.vector.tensor_scalar(
                        v8[:, tt, :], psv[:], S_V / (S_X * S_W), None, op0=OP.mult)

            # ---------- attention + out-proj + w1, interleaved ----------
            # qb-outer group order; once a qb's 8 head-groups are done, its
            # out-proj and w1 matmuls become PE filler work emitted between
            # the next groups (the ACT engine is the bottleneck during exp).
            NFT = MD // P      # 2 feature tiles of h1
            with tc.tile_pool(name="exs", bufs=2) as exs, \
                 tc.tile_pool(name="asml", bufs=2) as asml, \
                 tc.tile_pool(name="mw", bufs=1) as mw, \
                 tc.tile_pool(name="msml", bufs=1) as sml, \
                 tc.tile_pool(name="ps_sc", bufs=2, space="PSUM") as ps_sc, \
                 tc.tile_pool(name="ps_ctx", bufs=1, space="PSUM") as ps_ctx, \
                 tc.tile_pool(name="ps_dn", bufs=1, space="PSUM") as ps_dn, \
                 tc.tile_pool(name="ps_io", bufs=2, space="PSUM") as ps_io:
                h1p = mw.tile([P, NFT, SQ], bf, tag="h1p")
                h1sq = mw.tile([P, NFT, SQ], bf, tag="h1sq")
                h1n = mw.tile([P, NFT, SQ], bf, tag="h1n")
                stat = mw.tile([P, 3, SQ], f32, tag="stat")
                res_sb = mw.tile([P, NTT], f32, tag="res")
                nmean, work, m2r = stat[:, 0, :], stat[:, 1, :], stat[:, 2, :]
                ex2m = varm = rstd = work

                def emit_oproj(qb, dt):
                    qsl = slice(qb * 512, (qb + 1) * 512)
                    pso = ps_io.tile([P, 512], f32, tag="psio", name="pso")
                    for j in range(NC8 // 2):
                        nc.tensor.matmul(
                            pso[:],
                            lhsT=ow_sb[:, 2 * j:2 * j + 2, dt * P:(dt + 1) * P],
                            rhs=ctx8[:, 2 * j:2 * j + 2, qsl],
                            start=(j == 0), stop=(j == NC8 // 2 - 1),
                            perf_mode=DR)
                    # bias (incl. folded V bias) lives in b1_eff on host
                    nc.vector.tensor_scalar(
                        attbf[:, dt, qsl], pso[:],
                        1.0 / (S_CTX * S_W), None, op0=OP.mult)

                def emit_w1(qb, ft):
                    qsl = slice(qb * 512, (qb + 1) * 512)
                    psf = ps_io.tile([P, 512], f32, tag="psio", name="psf")
                    for c16 in range(2 * NC8):
                        if c16 < NC8:
                            rhs = xbf[:, c16, qsl]
                        else:
                            rhs = attbf[:, c16 - NC8, qsl]
                        nc.tensor.matmul(
                            psf[:],
                            lhsT=w1_sb[:, c16, ft * P:(ft + 1) * P],
                            rhs=rhs,
                            start=(c16 == 0), stop=(c16 == 2 * NC8 - 1))
                    nc.scalar.activation(
                        h1p[:, ft, qsl], psf[:],
                        ACT.Identity, bias=b1_sb[:, ft:ft + 1], scale=1.0)

                hb2_all = mw.tile([P, NTT, MD2], f32, tag="hb2_all")

                def emit_ln1(qb):
                    qsl = slice(qb * 512, (qb + 1) * 512)
                    for ft in range(NFT):
                        nc.vector.tensor_tensor(out=h1sq[:, ft, qsl],
                                                in0=h1p[:, ft, qsl],
                                                in1=h1p[:, ft, qsl], op=OP.mult)
                    psA = ps_io.tile([P, 512], f32, tag="psio", name="psA")
                    psB = ps_io.tile([P, 512], f32, tag="psio", name="psB")
                    for ft in range(NFT):
                        nc.tensor.matmul(psA[:], lhsT=onesbf[:],
                                         rhs=h1p[:, ft, qsl],
                                         start=(ft == 0), stop=(ft == NFT - 1))
                    for ft in range(NFT):
                        nc.tensor.matmul(psB[:], lhsT=onesbf[:],
                                         rhs=h1sq[:, ft, qsl],
                                         start=(ft == 0), stop=(ft == NFT - 1))
                    nc.vector.tensor_scalar_mul(nmean[:, qsl], psA[:], -1.0 / MD)
                    nc.vector.tensor_scalar_mul(ex2m[:, qsl], psB[:], 1.0 / MD)
                    nc.vector.tensor_tensor(out=m2r[:, qsl], in0=nmean[:, qsl],
                                            in1=nmean[:, qsl], op=OP.mult)
                    nc.vector.tensor_tensor(out=work[:, qsl], in0=work[:, qsl],
                                            in1=m2r[:, qsl], op=OP.subtract)
                    # rstd = exp(-0.5 * ln(var + eps))
                    nc.scalar.activation(varm[:, qsl], varm[:, qsl], ACT.Ln,
                                         bias=eps_sb[:, 0:1], scale=1.0)
                    nc.scalar.activation(rstd[:, qsl], varm[:, qsl], ACT.Exp,
                                         bias=0.0, scale=-0.5)
                    for ft in range(NFT):
                        nc.vector.tensor_tensor(out=h1n[:, ft, qsl],
                                                in0=h1p[:, ft, qsl],
                                                in1=nmean[:, qsl], op=OP.add)
                        nc.vector.tensor_tensor(out=h1n[:, ft, qsl],
                                                in0=h1n[:, ft, qsl],
                                                in1=rstd[:, qsl], op=OP.mult)
                        # relu on DVE (avoids an ACT table swap mid-exp-phase)
                        nc.vector.tensor_scalar(h1n[:, ft, qsl], h1n[:, ft, qsl],
                                                g1_sb[:, ft:ft + 1],
                                                be1_sb[:, ft:ft + 1],
                                                op0=OP.mult, op1=OP.add)
                        nc.vector.tensor_scalar_max(h1n[:, ft, qsl],
                                                    h1n[:, ft, qsl], 0.0)

                def emit_h2(tt, _unused=None):
                    ph2_t = ps_io.tile([P, 512], f32, tag="psio", name="ph2")
                    ph2 = ph2_t[:, :MD2]
                    for ft in range(MD // P):
                        nc.tensor.matmul(ph2, lhsT=h1n[:, ft, tt * P:(tt + 1) * P],
                                         rhs=w2_sb[:, ft, :],
                                         start=(ft == 0), stop=(ft == MD // P - 1))
                    nc.vector.scalar_tensor_tensor(out=hb2_all[:, tt, :], in0=ph2,
                                                   scalar=1.0, in1=b2_sb[:],
                                                   op0=OP.mult, op1=OP.add)

                F2 = float(MD2)

                def emit_tail(qb, _unused=None):
                    r = slice(qb * NTT // NQB, (qb + 1) * NTT // NQB)
                    nt = NTT // NQB
                    hb2 = hb2_all[:, r, :]
                    sums2 = sml.tile([P, NTT], f32, tag="sums2")
                    nc.vector.reduce_sum(sums2[:, r], hb2,
                                         axis=mybir.AxisListType.X)
                    msq = sml.tile([P, NTT, MD2], f32, tag="msq")
                    ssq2 = sml.tile([P, NTT], f32, tag="ssq2")
                    nc.vector.tensor_tensor(out=msq[:, r, :], in0=hb2,
                                            in1=hb2, op=OP.mult)
                    nc.vector.reduce_sum(ssq2[:, r], msq[:, r, :],
                                         axis=mybir.AxisListType.X)
                    nm2 = sml.tile([P, NTT], f32, tag="nm2")
                    nc.vector.tensor_scalar_mul(nm2[:, r], sums2[:, r], -1.0 / F2)
                    ex22 = sml.tile([P, NTT], f32, tag="ex22")
                    nc.vector.tensor_scalar_mul(ex22[:, r], ssq2[:, r], 1.0 / F2)
                    mm2 = sml.tile([P, NTT], f32, tag="mm2")
                    nc.vector.tensor_tensor(out=mm2[:, r], in0=nm2[:, r],
                                            in1=nm2[:, r], op=OP.mult)
                    var2 = sml.tile([P, NTT], f32, tag="var2")
                    nc.vector.tensor_tensor(out=var2[:, r], in0=ex22[:, r],
                                            in1=mm2[:, r], op=OP.subtract)
                    std2 = sml.tile([P, NTT], f32, tag="std2")
                    nc.scalar.activation(std2[:, r], var2[:, r], ACT.Ln,
                                         bias=eps_sb[:, 0:1], scale=1.0)
                    rstd2 = sml.tile([P, NTT], f32, tag="rstd2")
                    nc.scalar.activation(rstd2[:, r], std2[:, r], ACT.Exp,
                                         bias=0.0, scale=-0.5)
                    t1a = sml.tile([P, NTT, MD2], f32, tag="t1a")
                    nc.vector.tensor_tensor(
                        out=t1a[:, r, :], in0=hb2,
                        in1=nm2[:, r, None].to_broadcast([P, nt, MD2]), op=OP.add)
                    nc.vector.tensor_tensor(
                        out=t1a[:, r, :], in0=t1a[:, r, :],
                        in1=rstd2[:, r, None].to_broadcast([P, nt, MD2]),
                        op=OP.mult)
                    nc.vector.tensor_tensor(
                        out=t1a[:, r, :], in0=t1a[:, r, :],
                        in1=g2_sb[:, None, :].to_broadcast([P, nt, MD2]),
                        op=OP.mult)
                    nc.vector.tensor_tensor(
                        out=t1a[:, r, :], in0=t1a[:, r, :],
                        in1=be2_sb[:, None, :].to_broadcast([P, nt, MD2]),
                        op=OP.add)
                    nc.vector.tensor_scalar_max(t1a[:, r, :], t1a[:, r, :], 0.0)
                    nc.vector.tensor_tensor(
                        out=t1a[:, r, :], in0=t1a[:, r, :],
                        in1=w3_sb[:, None, :].to_broadcast([P, nt, MD2]),
                        op=OP.mult)
                    base8 = sml.tile([P, NTT], f32, tag="base8")
                    nc.vector.reduce_sum(base8[:, r], t1a[:, r, :],
                                         axis=mybir.AxisListType.X)
                    nc.vector.tensor_tensor(
                        out=base8[:, r], in0=base8[:, r],
                        in1=b3_sb[:, 0:1].to_broadcast([P, nt]), op=OP.add)
                    imp1a = sml.tile([P, NTT], f32, tag="imp1a")
                    nc.vector.tensor_scalar_add(imp1a[:, r], imp_all[:, r], 1.0)
                    nc.vector.tensor_tensor(out=base8[:, r], in0=base8[:, r],
                                            in1=imp1a[:, r], op=OP.mult)
                    nc.vector.tensor_scalar(base8[:, r], base8[:, r], MAX_W, MIN_W,
                                            op0=OP.min, op1=OP.max)
                    nc.vector.tensor_tensor(out=res_sb[:, r], in0=base8[:, r],
                                            in1=maskf_sb[:, r], op=OP.mult)
                    nc.sync.dma_start(
                        out[:].rearrange("(t p) -> p t", p=P)[:, r], res_sb[:, r])

                groups = [(h, qb) for qb in range(NQB) for h in range(NH)]
                NPAIR = NKT // 2
                fillers = []

                def emit_scexp(g, ex8):
                    h, qb = groups[g]
                    qsl = slice(qb * 512, (qb + 1) * 512)
                    for t in range(NPAIR):
                        scp = ps_sc.tile([P, 2, 512], f32, tag="scp")
                        for i in range(2):
                            nc.tensor.matmul(scp[:, i, :],
                                             lhsT=k8[:, h, (2 * t + i) * P:
                                                  (2 * t + i + 1) * P],
                                             rhs=q8[:, h, qsl],
                                             start=True, stop=True)
                        if 2 * t < nex_act:
                            nc.scalar.activation(ex8[:, 2 * t:2 * t + 2, :], scp[:],
                                                 ACT.Exp,
                                                 bias=kb_sb[:, 2 * t:2 * t + 1],
                                                 scale=1.0 / SCORE_PS)
                        else:
                            # Schraudolph exp straight to e4m3: uint8 convert
                            # saturates negatives to 0 (== zero attn weight)
                            nc.vector.tensor_scalar(
                                ex8[:, 2 * t:2 * t + 2, :].bitcast(u8),
                                scp[:], 8.0 * LOG2E / SCORE_PS,
                                kbs_sb[:, 2 * t:2 * t + 1],
                                op0=OP.mult, op1=OP.add)

                ex_tiles = {}

                def alloc_ex():
                    ex8 = exs.tile([P, NKT, 512], f8, tag="ex8", name="ex8")
                    return (ex8,)

                ex_tiles[0] = alloc_ex()
                emit_scexp(0, *ex_tiles[0])
                for g, (h, qb) in enumerate(groups):
                    qsl = slice(qb * 512, (qb + 1) * 512)
                    ex8, = ex_tiles.pop(g)
                    if g + 1 < len(groups):
                        ex_tiles[g + 1] = alloc_ex()
                        emit_scexp(g + 1, *ex_tiles[g + 1])
                    cps = ps_ctx.tile([P, 512], f32, tag="cps")
                    dn = ps_dn.tile([P, 512], f32, tag="dn")
                    n_mm = NKT // 2
                    for t in range(n_mm):
                        nc.tensor.matmul(cps[:],
                                         lhsT=v8[:, 2 * t:2 * t + 2, h * P:(h + 1) * P],
                                         rhs=ex8[:, 2 * t:2 * t + 2, :],
                                         start=(t == 0), stop=(t == n_mm - 1),
                                         perf_mode=DR)
                        nc.tensor.matmul(dn[:], lhsT=ones8[:],
                                         rhs=ex8[:, 2 * t:2 * t + 2, :],
                                         start=(t == 0), stop=(t == n_mm - 1),
                                         perf_mode=DR)
                    rcb = asml.tile([P, 512], f32, tag="rcb")
                    nc.vector.reciprocal_approx_fast(rcb[:], dn[:])
                    nc.vector.scalar_tensor_tensor(
                        out=ctx8[:, h, qsl], in0=cps[:], scalar=S_CTX / S_V,
                        in1=rcb[:], op0=OP.mult, op1=OP.mult)
                    if h == NH - 1:
                        fillers += [(emit_oproj, qb, dt) for dt in range(NC8)]
                        fillers += [(emit_w1, qb, ft) for ft in range(NFT)]
                        fillers += [(lambda q, _u: emit_ln1(q), qb, None)]
                        fillers += [(emit_h2, tt, None)
                                    for tt in range(qb * NTT // NQB,
                                                    (qb + 1) * NTT // NQB)]
                        fillers += [(emit_tail, qb, None)]
                    for _ in range(2):
                        if fillers:
                            fn, a, b2_ = fillers.pop(0)
                            fn(a, b2_)
                while fillers:
                    fn, a, b2_ = fillers.pop(0)
                    fn(a, b2_)

                # LN2/final emitted per-qb as fillers (emit_tail)

    nc.compile()
    return nc


def _get_program():
    nex_act = int(os.environ.get("KB_NEX_ACT", "14"))
    key = ("nc", nex_act)
    if key not in _CACHE:
        _CACHE[key] = _build(nex_act)
    return _CACHE[key]


def _prep_in_maps(inputs):
    import ml_dtypes
    bf16 = ml_dtypes.bfloat16
    f8 = ml_dtypes.float8_e4m3

    hidden = np.asarray(inputs["hidden_states"], dtype=np.float32)
    token_ids = np.asarray(inputs["token_ids"], dtype=np.int32)
    mask = np.asarray(inputs["attention_mask"]).astype(bool)
    pos = np.asarray(inputs["pos_embed"], dtype=np.float32)
    in_proj_w = np.asarray(inputs["in_proj_w"], dtype=np.float32)
    in_proj_b = np.asarray(inputs["in_proj_b"], dtype=np.float32)
    out_w = np.asarray(inputs["out_w"], dtype=np.float32)
    out_b = np.asarray(inputs["out_b"], dtype=np.float32)
    w1 = np.asarray(inputs["w1"], dtype=np.float32)
    b1 = np.asarray(inputs["b1"], dtype=np.float32)
    g1 = np.asarray(inputs["g1"], dtype=np.float32)
    beta1 = np.asarray(inputs["beta1"], dtype=np.float32)
    w2 = np.asarray(inputs["w2"], dtype=np.float32)
    b2 = np.asarray(inputs["b2"], dtype=np.float32)
    g2 = np.asarray(inputs["g2"], dtype=np.float32)
    beta2 = np.asarray(inputs["beta2"], dtype=np.float32)
    w3 = np.asarray(inputs["w3"], dtype=np.float32)
    b3 = np.asarray(inputs["b3"], dtype=np.float32)
    table = np.asarray(inputs["importance_table"], dtype=np.float32)

    B, S_, H_ = hidden.shape
    assert (B, S_, H_) == (4, S, H), (B, S_, H_)

    posT = np.ascontiguousarray(pos[0].T)                      # [H, S]
    wqT = in_proj_w[0:H].T
    wkT = in_proj_w[H:2 * H].T
    wvT = in_proj_w[2 * H:3 * H].T
    bq = in_proj_b[0:H]
    bk = in_proj_b[H:2 * H]
    bv = in_proj_b[2 * H:3 * H]
    owT = out_w.T
    # softmax weights sum to 1, so the V bias passes through attention as a
    # constant: fold it (and out_b) into the out-proj bias, then fold that
    # into the w1 bias (b1_eff), since attended only feeds w1.
    ob_eff = bv @ out_w.T + out_b
    b1_eff = b1 + w1[:, H:2 * H] @ ob_eff

    def warr(wT, s=S_W):   # [H, N] -> [128, 8, N] fp8 scaled
        return np.ascontiguousarray(
            np.clip(wT * s, -224.0, 224.0)
            .reshape(NC8, P, -1).transpose(1, 0, 2)).astype(f8)

    def cmaj(v, s=1.0):   # [F] -> [128, F/128]
        return np.ascontiguousarray((v * s).reshape(-1, P).T.astype(np.float32))

    def bcast(v):  # [F] -> [128, F]
        return np.ascontiguousarray(
            np.broadcast_to(v[None, :], (P, v.shape[0])).astype(np.float32))

    shared = {
        "wq8": warr(wqT), "wk8": warr(wkT),
        "wv8": warr(wvT), "ow8": warr(owT),
        # x is stored on-chip as 16x (bf16); compensate in w1's x-half
        "w1bf": np.ascontiguousarray(
            np.concatenate([w1.T[:H] / S_X, w1.T[H:]], axis=0)
            .reshape(2 * NC8, P, MD).transpose(1, 0, 2)).astype(bf16),
        "w2bf": np.ascontiguousarray(
            w2.T.reshape(MD // P, P, MD2).transpose(1, 0, 2)).astype(bf16),
        "bq_c": cmaj(bq, S_Q / np.sqrt(HD)),
        "b1_c": cmaj(b1_eff), "g1_c": cmaj(g1), "be1_c": cmaj(beta1),
        "b2_b": bcast(b2), "g2_b": bcast(g2), "be2_b": bcast(beta2),
        "w3_b": bcast(w3[0]), "b3_c": np.full((P, 1), b3[0], dtype=np.float32),
        "table": np.ascontiguousarray(table[:, None]),
    }
    in_maps = []
    for c in range(8):
        b = c // 2
        half = c % 2
        own = slice(half * SQ, (half + 1) * SQ)
        oth = slice((1 - half) * SQ, (2 - half) * SQ)
        hT_b = hidden[b].T * S_X
        posT_s = posT * S_X
        hT_arr = np.ascontiguousarray(
            np.concatenate([hT_b[:, own], hT_b[:, oth]], axis=1)).astype(bf16)
        pT_arr = np.ascontiguousarray(
            np.concatenate([posT_s[:, own], posT_s[:, oth]], axis=1)).astype(bf16)
        kb = np.where(mask[b], 0.0, -1e9).astype(np.float32)
        kb_arr = np.concatenate([kb[own], kb[oth]]) - C_SHIFT
        kbs_arr = (56.0 - 0.5) + 8.0 * LOG2E * (
            np.concatenate([kb[own], kb[oth]]) - C_SHIFT)
        m = {
            "hT": hT_arr, "pT": pT_arr,
            "kb_c": np.ascontiguousarray(kb_arr.reshape(-1, P).T),
            "kbs_c": np.ascontiguousarray(
                kbs_arr.reshape(-1, P).T.astype(np.float32)),
            "maskf": np.ascontiguousarray(
                mask[b, own].astype(np.float32).reshape(-1, P).T),
            "tok": np.ascontiguousarray(token_ids[b, own][:, None]),
        }
        m.update(shared)
        in_maps.append(m)
    return in_maps


def _assemble(res):
    full = np.zeros((4, S), dtype=np.float32)
    for c in range(8):
        b = c // 2
        half = c % 2
        full[b, half * SQ:(half + 1) * SQ] = res.results[c]["out"]
    return full


def kernel(**inputs) -> np.ndarray:
    from concourse.bass_utils import run_bass_kernel_spmd
    in_maps = _prep_in_maps(inputs)
    nc = _get_program()
    res = run_bass_kernel_spmd(nc, in_maps, list(range(8)))
    return _assemble(res)


def run_traced(inputs, **kwargs):
    from concourse.bass_utils import run_bass_kernel_spmd
    in_maps = _prep_in_maps(inputs)
    nc = _get_program()
    return run_bass_kernel_spmd(nc, in_maps, list(range(8)), trace=True, **kwargs)
